# revision 10
# baseline (speedup 1.0000x reference)
"""CapsNet forward pass on 8 Trainium2 NeuronCores (Bass/Tile).

Data-parallel: batch 512 sharded 64/core; parameters replicated; the routing
b_ij batch-mean update is an AllReduce. Host-side prep is layout-only
(im2col of the input, weight transposes) - all FLOPs run on device.

Self-contained: hardcodes all shapes from the problem spec.
"""
import os
import numpy as np

import concourse.bacc as bacc
import concourse.tile as tile
from concourse import bass_utils, mybir

F32 = mybir.dt.float32
F32R = mybir.dt.float32r
AF = mybir.ActivationFunctionType
ALU = mybir.AluOpType
AXL = mybir.AxisListType

N_CORES = 8
B = 64            # batch per core
BH = 32           # batch per conv sub-pass
R = 1152          # num routes
C = 10            # num capsules
O = 16            # out dim
RI = R * 8        # 9216
CO = C * O        # 160
KT = RI // 128    # 72 k-tiles over (r,i)
NUM_ITERS = 3

DEBUG = bool(int(os.environ.get("BASS_CAPS_DEBUG", "0")))

_CACHED_NC = None


def build():
    nc = bacc.Bacc("TRN2", target_bir_lowering=False, debug=False,
                   num_devices=N_CORES)

    # ---------------- DRAM I/O ----------------
    d_im2col = nc.dram_tensor("im2col", [81, 20, B, 20], F32, kind="ExternalInput")
    d_cwT = nc.dram_tensor("cwT", [81, 256], F32, kind="ExternalInput")
    d_conv_b = nc.dram_tensor("conv_b", [256, 1], F32, kind="ExternalInput")
    d_pcwT = nc.dram_tensor("pcwT", [2, 81, 128, 2, 128], F32, kind="ExternalInput")
    d_pc_b = nc.dram_tensor("pc_b", [256, 1], F32, kind="ExternalInput")
    d_Wmm = nc.dram_tensor("Wmm", [RI, CO], F32, kind="ExternalInput")
    d_S2 = nc.dram_tensor("S2", [128, 16], F32, kind="ExternalInput")
    d_ident = nc.dram_tensor("ident", [128, 128], F32, kind="ExternalInput")
    d_w1T = nc.dram_tensor("w1T", [CO, 512], F32, kind="ExternalInput")
    d_b1 = nc.dram_tensor("b1", [512, 1], F32, kind="ExternalInput")
    d_w2T = nc.dram_tensor("w2T", [512, 10], F32, kind="ExternalInput")
    d_b2rep = nc.dram_tensor("b2rep", [B, 9], F32, kind="ExternalInput")
    d_out = nc.dram_tensor("out", [B, 9], F32, kind="ExternalOutput")

    dbg = {}
    if DEBUG:
        dbg["xact"] = nc.dram_tensor("dbg_xact", [2, 128, 20, BH, 20], F32, kind="ExternalOutput")
        dbg["x2s"] = nc.dram_tensor("dbg_x2s", [2, 128, 6, B, 6], F32, kind="ExternalOutput")
        dbg["u"] = nc.dram_tensor("dbg_u", [B, RI], F32, kind="ExternalOutput")
        dbg["s0T"] = nc.dram_tensor("dbg_s0T", [128, B], F32, kind="ExternalOutput")
        dbg["db0"] = nc.dram_tensor("dbg_db0", [C, R], F32, kind="ExternalOutput")
        dbg["c1T"] = nc.dram_tensor("dbg_c1T", [C, R], F32, kind="ExternalOutput")
        dbg["vT2"] = nc.dram_tensor("dbg_vT2", [CO, B], F32, kind="ExternalOutput")
        dbg["hT"] = nc.dram_tensor("dbg_hT", [128, 4, B], F32, kind="ExternalOutput")

    with (
        tile.TileContext(nc) as tc,
        tc.tile_pool(name="persist", bufs=1) as P,
        tc.tile_pool(name="dram", bufs=1, space="DRAM") as DR,
    ):
        # ====== constants / small weights ======
        cw = P.tile([81, 256], F32R, tag="cw")
        nc.gpsimd.dma_start(cw[:], d_cwT[:])
        cb0 = P.tile([128, 2], F32, tag="cb0")
        nc.sync.dma_start(cb0[:, 0:1], d_conv_b[0:128, :])
        nc.sync.dma_start(cb0[:, 1:2], d_conv_b[128:256, :])
        pb0 = P.tile([128, 2], F32, tag="pb0")
        nc.sync.dma_start(pb0[:, 0:1], d_pc_b[0:128, :])
        nc.sync.dma_start(pb0[:, 1:2], d_pc_b[128:256, :])
        ident = P.tile([128, 128], F32, tag="ident")
        nc.sync.dma_start(ident[:], d_ident[:])
        S2 = P.tile([128, 16], F32R, tag="S2")
        nc.gpsimd.dma_start(S2[:], d_S2[:])

        x2s = [P.tile([128, 6, B, 6], F32, tag=f"x2s{mt}", name=f"x2s{mt}") for mt in range(2)]

        # ================= conv phase (per batch half) =================
        with (
            tc.tile_pool(name="convsb", bufs=1) as CB,
            tc.tile_pool(name="imcpool", bufs=3) as IMC,
            tc.tile_pool(name="wstream", bufs=4) as WS,
            tc.tile_pool(name="ps_c1", bufs=2, space="PSUM") as PC1,
            tc.tile_pool(name="ps_c2", bufs=1, space="PSUM") as PC2,
        ):
            xact = [CB.tile([128, 20, BH, 20], F32R, tag=f"xact{mt}", name=f"xact{mt}")
                    for mt in range(2)]
            for bh in range(2):
                bsl = slice(bh * BH, (bh + 1) * BH)
                # ---- conv1 ----
                for y in range(20):
                    imc = IMC.tile([81, BH, 20], F32R, tag="imc")
                    nc.gpsimd.dma_start(imc[:], d_im2col[:, y, bsl, :])
                    for mt in range(2):
                        ps = PC1.tile([128, BH * 20], F32, tag="c1ps")
                        for j, (n0, n1) in enumerate(((0, 384), (384, 640),)):
                            nc.tensor.matmul(
                                ps[:, n0:n1],
                                cw[:, mt * 128:(mt + 1) * 128],
                                imc[:].rearrange("p a b -> p (a b)")[:, n0:n1],
                                start=(j == 0), stop=(j == 1),
                            )
                        dst = xact[mt][:, y, :, :].rearrange("p a b -> p (a b)")
                        if mt == 0:
                            nc.vector.tensor_scalar(
                                dst, ps[:], cb0[:, 0:1], 0.0, ALU.add, ALU.max)
                        else:
                            nc.scalar.activation(
                                dst, ps[:], AF.Relu, bias=cb0[:, 1:2], scale=1.0)
                if DEBUG and bh == 1:
                    for mt in range(2):
                        nc.sync.dma_start(dbg["xact"][mt], xact[mt][:].bitcast(F32))

                # ---- conv2 ----
                for mt in range(2):
                    ps2 = [PC2.tile([128, 2, BH, 6], F32, tag=f"c2ps{j}", name=f"c2ps{j}")
                           for j in range(3)]
                    for dydx in range(81):
                        dy, dx = divmod(dydx, 9)
                        wt = WS.tile([128, 2, 128], F32R, tag="wt")
                        nc.gpsimd.dma_start(wt[:], d_pcwT[mt, dydx])
                        for kh in range(2):
                            for oyp in range(3):
                                # rows 4*oyp + 2j + dy, j in {0,1}
                                rhs = xact[kh][:, 4 * oyp + dy:4 * oyp + dy + 3:2,
                                               :, dx:dx + 11:2]
                                nc.tensor.matmul(
                                    ps2[oyp][:], wt[:, kh, :], rhs,
                                    start=(dydx == 0 and kh == 0),
                                    stop=(dydx == 80 and kh == 1),
                                )
                    for oyp in range(3):
                        nc.vector.tensor_scalar(
                            x2s[mt][:, 2 * oyp:2 * oyp + 2, bsl, :],
                            ps2[oyp][:], pb0[:, mt:mt + 1], None, ALU.add,
                        )
        if DEBUG:
            for mt in range(2):
                nc.sync.dma_start(dbg["x2s"][mt], x2s[mt][:])

        # ================= u phase =================
        with tc.tile_pool(name="upool", bufs=1) as UP:
            u_byB = UP.tile([B, RI], F32R, tag="u_byB")
            uT = UP.tile([128, KT, B], F32R, tag="uT")

            with (
                tc.tile_pool(name="usc", bufs=1) as USC,
                tc.tile_pool(name="usc2", bufs=2) as USC2,
                tc.tile_pool(name="ps_tr", bufs=4, space="PSUM") as PTR,
            ):
                u_pre = USC.tile([B, RI], F32, tag="u_pre")
                for mt in range(2):
                    for oy in range(6):
                        for ox in range(6):
                            tp = PTR.tile([128, 128], F32, tag="tr")
                            nc.tensor.transpose(
                                tp[0:B, :], x2s[mt][:, oy, :, ox], ident[:])
                            dst = u_pre[:, mt * 4608 + oy * 6 + ox::36][:, 0:128]
                            nc.vector.tensor_copy(dst, tp[0:B, :])

                sn = USC.tile([B, R], F32, tag="sn")
                for ch in range(9):
                    sl = slice(ch * 1024, (ch + 1) * 1024)
                    sq = USC2.tile([B, 1024], F32, tag="sq")
                    nc.scalar.activation(sq[:], u_pre[:, sl], AF.Square)
                    nc.vector.tensor_reduce(
                        sn[:, ch * 128:(ch + 1) * 128],
                        sq[:].rearrange("p (g i) -> p g i", i=8),
                        AXL.X, ALU.add,
                    )
                fac = USC.tile([B, R], F32, tag="fac")
                rt = USC.tile([B, R], F32, tag="rt")
                nc.scalar.activation(rt[:], sn[:], AF.Sqrt)
                snp = USC.tile([B, R], F32, tag="snp")
                nc.vector.tensor_scalar(snp[:], sn[:], 1.0, None, ALU.add)
                rsnp = USC.tile([B, R], F32, tag="rsnp")
                nc.vector.reciprocal(rsnp[:], snp[:])
                nc.vector.tensor_tensor(fac[:], rt[:], rsnp[:], ALU.mult)
                nc.vector.tensor_tensor(
                    u_byB[:].rearrange("p (r i) -> p r i", i=8),
                    u_pre[:].rearrange("p (r i) -> p r i", i=8),
                    fac[:].unsqueeze(2).to_broadcast([B, R, 8]),
                    ALU.mult,
                )
                if DEBUG:
                    nc.sync.dma_start(dbg["u"][:], u_byB[:].bitcast(F32))
                for k in range(KT):
                    tp = PTR.tile([128, 128], F32, tag="tr")
                    nc.tensor.transpose(
                        tp[:, 0:B], u_byB[:, k * 128:(k + 1) * 128].bitcast(F32),
                        ident[0:B, 0:B])
                    nc.vector.tensor_copy(uT[:, k, :], tp[:, 0:B])

            # ================= routing phase =================
            vT0 = P.tile([128, B], F32R, tag="vT0")
            vT1 = P.tile([32, B], F32R, tag="vT1")
            with (
                tc.tile_pool(name="rt_big", bufs=1) as RB,
                tc.tile_pool(name="rt_sc", bufs=1) as RS,
                tc.tile_pool(name="rt_sc2", bufs=2) as RS2,
                tc.tile_pool(name="ps_st", bufs=1, space="PSUM") as PST,
                tc.tile_pool(name="ps_m", bufs=2, space="PSUM") as PSM,
                tc.tile_pool(name="ps_db", bufs=1, space="PSUM") as PDB,
                tc.tile_pool(name="ps_tp", bufs=1, space="PSUM") as PTP,
            ):
                Wmm = RB.tile([128, KT, CO], F32R, tag="Wmm")
                nc.gpsimd.dma_start(
                    Wmm[:], d_Wmm[:].rearrange("(k p) c -> p k c", p=128))
                W1mm = RB.tile([128, KT, CO], F32R, tag="W1mm")
                c_rep = RB.tile([128, KT, C], F32, tag="c_rep")
                bacc_t = RB.tile([C, R], F32, tag="bacc")
                c_T = RB.tile([C, R], F32, tag="c_T")
                c_byR = RB.tile([128, 9, C], F32, tag="c_byR")

                d_crdram = DR.tile([R, C], F32)
                d_red_in = DR.tile([C, R], F32)
                d_red_out = DR.tile([C, R], F32)

                for it in range(NUM_ITERS):
                    # ---- s_t = W'^T @ u ----
                    st0 = PST.tile([128, B], F32, tag="st0")
                    st1 = PST.tile([32, B], F32, tag="st1")
                    lhs = Wmm if it == 0 else W1mm
                    for k in range(KT):
                        nc.tensor.matmul(st0[:], lhs[:, k, 0:128], uT[:, k, :],
                                         start=(k == 0), stop=(k == KT - 1))
                    for k in range(KT):
                        nc.tensor.matmul(st1[:], lhs[:, k, 128:160], uT[:, k, :],
                                         start=(k == 0), stop=(k == KT - 1))
                    # ---- v = squash(s) elementwise ----
                    for half, (st, vt, np_) in enumerate(
                            ((st0, vT0, 128), (st1, vT1, 32))):
                        s_sb = RS2.tile([np_, B], F32, tag=f"s_sb{half}")
                        if it == 0:
                            nc.vector.tensor_scalar(
                                s_sb[:], st[:], 1.0 / R, None, ALU.mult)
                        else:
                            nc.vector.tensor_copy(s_sb[:], st[:])
                        if DEBUG and it == 0 and half == 0:
                            nc.sync.dma_start(dbg["s0T"][:], s_sb[:])
                        t2 = RS2.tile([np_, B], F32, tag=f"t2_{half}")
                        nc.vector.tensor_tensor(t2[:], s_sb[:], s_sb[:], ALU.mult)
                        num = RS2.tile([np_, B], F32, tag=f"num{half}")
                        nc.vector.tensor_tensor(num[:], t2[:], s_sb[:], ALU.mult)
                        rte = RS2.tile([np_, B], F32, tag=f"rte{half}")
                        nc.scalar.activation(rte[:], t2[:], AF.Sqrt)
                        den = RS2.tile([np_, B], F32, tag=f"den{half}")
                        nc.vector.tensor_scalar(den[:], t2[:], 1.0, None, ALU.add)
                        nc.vector.tensor_tensor(den[:], den[:], rte[:], ALU.mult)
                        rden = RS2.tile([np_, B], F32, tag=f"rden{half}")
                        nc.vector.reciprocal(rden[:], den[:])
                        nc.vector.tensor_tensor(vt[:], num[:], rden[:], ALU.mult)

                    if it == NUM_ITERS - 1:
                        break

                    # ---- v -> [B, CO] ----
                    v_b = RS.tile([B, CO], F32R, tag="v_b")
                    tpv0 = PTP.tile([128, 128], F32, tag="tp")
                    nc.tensor.transpose(tpv0[0:B, :], vT0[:].bitcast(F32), ident[:])
                    nc.vector.tensor_copy(v_b[:, 0:128], tpv0[0:B, :])
                    tpv1 = PTP.tile([128, 128], F32, tag="tp")
                    nc.tensor.transpose(tpv1[0:B, 0:32], vT1[:].bitcast(F32), ident[0:32, 0:32])
                    nc.vector.tensor_copy(v_b[:, 128:160], tpv1[0:B, 0:32])

                    # ---- db^T[c, r] = sum_oi W*(u^T v)/Btot via per-ktile chain ----
                    db_ps = PDB.tile([C, R], F32, tag="db_ps")
                    for k in range(KT):
                        Mps = PSM.tile([128, CO], F32, tag="Mps")
                        nc.tensor.matmul(
                            Mps[:], u_byB[:, k * 128:(k + 1) * 128], v_b[:],
                            start=True, stop=True)
                        prod = RS2.tile([128, CO], F32, tag="prod")
                        nc.vector.tensor_tensor(
                            prod[:], Mps[:], Wmm[:, k, :], ALU.mult)
                        Tk = RS2.tile([128, C], F32R, tag="Tk")
                        with nc.allow_low_precision(reason="f32r rounding for PE"):
                            nc.vector.tensor_reduce(
                                Tk[:], prod[:].rearrange("p (c o) -> p c o", o=O),
                                AXL.X, ALU.add)
                        nc.tensor.matmul(
                            db_ps[:, k * 16:(k + 1) * 16], Tk[:], S2[:],
                            start=True, stop=True)
                    db_sb = RS.tile([C, R], F32, tag="db_sb")
                    nc.vector.tensor_copy(db_sb[:], db_ps[:])

                    # ---- AllReduce batch-mean across cores ----
                    nc.sync.dma_start(d_red_in[:], db_sb[:])
                    nc.gpsimd.collective_compute(
                        "AllReduce", ALU.add,
                        replica_groups=[list(range(N_CORES))],
                        ins=[d_red_in.opt()], outs=[d_red_out.opt()],
                    )
                    db_red = RS.tile([C, R], F32, tag="db_red")
                    nc.sync.dma_start(db_red[:], d_red_out[:])
                    if it == 0:
                        nc.vector.tensor_copy(bacc_t[:], db_red[:])
                        if DEBUG:
                            nc.sync.dma_start(dbg["db0"][:], bacc_t[:])
                    else:
                        nc.vector.tensor_tensor(
                            bacc_t[:], bacc_t[:], db_red[:], ALU.add)

                    # ---- c = softmax_r(b) on [C, R] ----
                    mx = RS.tile([C, 1], F32, tag="mx")
                    nc.vector.tensor_reduce(mx[:], bacc_t[:], AXL.X, ALU.max)
                    nmx = RS.tile([C, 1], F32, tag="nmx")
                    nc.vector.tensor_scalar(nmx[:], mx[:], -1.0, None, ALU.mult)
                    ex = RS.tile([C, R], F32, tag="ex")
                    nc.scalar.activation(ex[:], bacc_t[:], AF.Exp,
                                         bias=nmx[:], scale=1.0)
                    sm = RS.tile([C, 1], F32, tag="sm")
                    nc.vector.tensor_reduce(sm[:], ex[:], AXL.X, ALU.add)
                    rcp = RS.tile([C, 1], F32, tag="rcp")
                    nc.vector.reciprocal(rcp[:], sm[:])
                    nc.vector.tensor_scalar(c_T[:], ex[:], rcp[:], None, ALU.mult)
                    if DEBUG and it == 0:
                        nc.sync.dma_start(dbg["c1T"][:], c_T[:])

                    # ---- c_rep[(r16,i),(k,c)] via DRAM round trip ----
                    for rb in range(9):
                        tpc = PTP.tile([128, 128], F32, tag="tp")
                        nc.tensor.transpose(
                            tpc[:, 0:C], c_T[:, rb * 128:(rb + 1) * 128],
                            ident[0:C, 0:C])
                        nc.vector.tensor_copy(c_byR[:, rb, :], tpc[:, 0:C])
                    nc.sync.dma_start(
                        d_crdram[:].rearrange("(a p) c -> p a c", p=128),
                        c_byR[:])
                    for rlo in range(16):
                        nc.sync.dma_start(
                            c_rep[rlo * 8:(rlo + 1) * 8, :, :],
                            d_crdram[:].rearrange("(k rl) c -> rl k c", rl=16)[rlo]
                            .unsqueeze(0).to_broadcast([8, KT, C]),
                        )
                    # ---- W' = W * c (broadcast over o) ----
                    nc.vector.tensor_tensor(
                        W1mm[:].rearrange("p k (c o) -> p k c o", o=O),
                        Wmm[:].rearrange("p k (c o) -> p k c o", o=O),
                        c_rep[:].unsqueeze(3).to_broadcast([128, KT, C, O]),
                        ALU.mult,
                    )

        # ================= encoder =================
        if DEBUG:
            nc.sync.dma_start(dbg["vT2"][0:128, :], vT0[:].bitcast(F32))
            nc.sync.dma_start(dbg["vT2"][128:160, :], vT1[:].bitcast(F32))
        with (
            tc.tile_pool(name="enc", bufs=1) as EN,
            tc.tile_pool(name="enc2", bufs=2) as EN2,
            tc.tile_pool(name="ps_h", bufs=2, space="PSUM") as PSH,
            tc.tile_pool(name="ps_o", bufs=1, space="PSUM") as PSO,
        ):
            w1T0 = EN.tile([128, 512], F32R, tag="w1T0")
            nc.gpsimd.dma_start(w1T0[:], d_w1T[0:128, :])
            w1T1 = EN.tile([32, 512], F32R, tag="w1T1")
            nc.gpsimd.dma_start(w1T1[:], d_w1T[128:160, :])
            b1sb = EN.tile([128, 4], F32, tag="b1sb")
            for jt in range(4):
                nc.sync.dma_start(b1sb[:, jt:jt + 1],
                                  d_b1[jt * 128:(jt + 1) * 128, :])
            w2T = EN.tile([128, 4, 10], F32R, tag="w2T")
            nc.gpsimd.dma_start(
                w2T[:], d_w2T[:].rearrange("(j p) c -> p j c", p=128))
            b2sb = EN.tile([B, 9], F32, tag="b2sb")
            nc.sync.dma_start(b2sb[:], d_b2rep[:])

            hT = EN.tile([128, 4, B], F32R, tag="hT")
            for jt in range(4):
                hp = PSH.tile([128, B], F32, tag="hp")
                nc.tensor.matmul(hp[:], w1T0[:, jt * 128:(jt + 1) * 128], vT0[:],
                                 start=True, stop=False)
                nc.tensor.matmul(hp[:], w1T1[:, jt * 128:(jt + 1) * 128], vT1[:],
                                 start=False, stop=True)
                nc.scalar.activation(hT[:, jt, :], hp[:], AF.Relu,
                                     bias=b1sb[:, jt:jt + 1], scale=1.0)
            if DEBUG:
                nc.sync.dma_start(dbg["hT"][:], hT[:].bitcast(F32))

            op = PSO.tile([B, 10], F32, tag="op")
            for jt in range(4):
                nc.tensor.matmul(op[:], hT[:, jt, :], w2T[:, jt, :],
                                 start=(jt == 0), stop=(jt == 3))
            logit = EN2.tile([B, 9], F32, tag="logit")
            nc.vector.tensor_tensor(logit[:], op[:, 0:9], b2sb[:], ALU.add)
            out_sb = EN2.tile([B, 9], F32, tag="out_sb")
            nc.scalar.activation(out_sb[:], logit[:], AF.Sigmoid)
            nc.sync.dma_start(d_out[:], out_sb[:])

    nc.compile()
    return nc


def _host_prep(data, conv_w, conv_b, pc_w, pc_b, W, enc_w1, enc_b1, enc_w2, enc_b2):
    """Layout-only host prep. Returns (shared_inputs, per_core_im2col)."""
    Bfull = data.shape[0]
    assert Bfull == N_CORES * B
    d = np.ascontiguousarray(data[:, 0])  # [512, 28, 28]
    sw = np.lib.stride_tricks.sliding_window_view(d, (9, 9), axis=(1, 2))
    # sw: [Bfull, 20, 20, 9, 9] -> (dy,dx,y,b,x)
    im2col_all = np.ascontiguousarray(sw.transpose(3, 4, 1, 0, 2)).reshape(
        81, 20, Bfull, 20)
    per_core = [np.ascontiguousarray(im2col_all[:, :, c * B:(c + 1) * B, :])
                for c in range(N_CORES)]

    cwT = np.ascontiguousarray(conv_w.reshape(256, 81).T)          # [81, 256]
    pcwT = np.ascontiguousarray(
        pc_w.reshape(2, 128, 2, 128, 81).transpose(0, 4, 3, 2, 1))  # [2,81,128,2,128]
    Wmm = np.ascontiguousarray(W.transpose(0, 3, 1, 2).reshape(RI, CO))
    S2 = np.zeros((128, 16), dtype=np.float32)
    for rlo in range(16):
        for i in range(8):
            S2[rlo * 8 + i, rlo] = 1.0 / (N_CORES * B)
    shared = dict(
        cwT=cwT,
        conv_b=np.asarray(conv_b, np.float32).reshape(256, 1),
        pcwT=pcwT,
        pc_b=np.asarray(pc_b, np.float32).reshape(256, 1),
        Wmm=Wmm,
        S2=S2,
        ident=np.eye(128, dtype=np.float32),
        w1T=np.ascontiguousarray(np.asarray(enc_w1, np.float32).T),   # [160, 512]
        b1=np.asarray(enc_b1, np.float32).reshape(512, 1),
        w2T=np.ascontiguousarray(np.pad(np.asarray(enc_w2, np.float32).T,
                                        ((0, 0), (0, 1)))),          # [512, 10]
        b2rep=np.tile(np.asarray(enc_b2, np.float32).reshape(1, 9), (B, 1)),
    )
    return shared, per_core


def kernel(**inputs):
    global _CACHED_NC
    if _CACHED_NC is None:
        _CACHED_NC = build()
    nc = _CACHED_NC
    inputs = {k: np.asarray(v, dtype=np.float32) for k, v in inputs.items()}
    shared, per_core = _host_prep(**inputs)
    in_maps = [dict(shared, im2col=per_core[c]) for c in range(N_CORES)]
    res = bass_utils.run_bass_kernel_spmd(nc, in_maps, core_ids=list(range(N_CORES)))
    out = np.concatenate([res.results[c]["out"] for c in range(N_CORES)], axis=0)
    return out


if __name__ == "__main__":
    import reference
    inputs = {k: np.asarray(v) for k, v in reference.setup_inputs().items()}
    got = kernel(**inputs)
    exp = np.asarray(reference.reference(**inputs))
    rel = np.abs(got - exp).max() / np.abs(exp).max()
    print("Relative error:", rel)


# revision 11
# speedup vs baseline: 1.0018x; 1.0018x over previous
"""CapsNet forward pass on 8 Trainium2 NeuronCores (Bass/Tile).

Data-parallel: batch 512 sharded 64/core; parameters replicated; the routing
b_ij batch-mean update is an AllReduce. Host-side prep is layout-only
(im2col of the input, weight transposes) - all FLOPs run on device.

Self-contained: hardcodes all shapes from the problem spec.
"""
import os
import numpy as np

import concourse.bacc as bacc
import concourse.tile as tile
from concourse import bass_utils, mybir

F32 = mybir.dt.float32
F32R = mybir.dt.float32r
AF = mybir.ActivationFunctionType
ALU = mybir.AluOpType
AXL = mybir.AxisListType

N_CORES = 8
B = 64            # batch per core
BH = 32           # batch per conv sub-pass
R = 1152          # num routes
C = 10            # num capsules
O = 16            # out dim
RI = R * 8        # 9216
CO = C * O        # 160
KT = RI // 128    # 72 k-tiles over (r,i)
NUM_ITERS = 3

DEBUG = bool(int(os.environ.get("BASS_CAPS_DEBUG", "0")))

_CACHED_NC = None


def build():
    nc = bacc.Bacc("TRN2", target_bir_lowering=False, debug=False,
                   num_devices=N_CORES)

    # ---------------- DRAM I/O ----------------
    d_im2col = nc.dram_tensor("im2col", [81, 20, B, 20], F32, kind="ExternalInput")
    d_cwT = nc.dram_tensor("cwT", [81, 256], F32, kind="ExternalInput")
    d_conv_b = nc.dram_tensor("conv_b", [256, 1], F32, kind="ExternalInput")
    d_pcwT = nc.dram_tensor("pcwT", [2, 81, 128, 2, 128], F32, kind="ExternalInput")
    d_pc_b = nc.dram_tensor("pc_b", [256, 1], F32, kind="ExternalInput")
    d_Wmm = nc.dram_tensor("Wmm", [RI, CO], F32, kind="ExternalInput")
    d_S2 = nc.dram_tensor("S2", [128, 16], F32, kind="ExternalInput")
    d_ident = nc.dram_tensor("ident", [128, 128], F32, kind="ExternalInput")
    d_w1T = nc.dram_tensor("w1T", [CO, 512], F32, kind="ExternalInput")
    d_b1 = nc.dram_tensor("b1", [512, 1], F32, kind="ExternalInput")
    d_w2T = nc.dram_tensor("w2T", [512, 10], F32, kind="ExternalInput")
    d_b2rep = nc.dram_tensor("b2rep", [B, 9], F32, kind="ExternalInput")
    d_out = nc.dram_tensor("out", [B, 9], F32, kind="ExternalOutput")

    dbg = {}
    if DEBUG:
        dbg["xact"] = nc.dram_tensor("dbg_xact", [2, 128, 20, BH, 20], F32, kind="ExternalOutput")
        dbg["x2s"] = nc.dram_tensor("dbg_x2s", [2, 128, 6, B, 6], F32, kind="ExternalOutput")
        dbg["u"] = nc.dram_tensor("dbg_u", [B, RI], F32, kind="ExternalOutput")
        dbg["s0T"] = nc.dram_tensor("dbg_s0T", [128, B], F32, kind="ExternalOutput")
        dbg["db0"] = nc.dram_tensor("dbg_db0", [C, R], F32, kind="ExternalOutput")
        dbg["c1T"] = nc.dram_tensor("dbg_c1T", [C, R], F32, kind="ExternalOutput")
        dbg["vT2"] = nc.dram_tensor("dbg_vT2", [CO, B], F32, kind="ExternalOutput")
        dbg["hT"] = nc.dram_tensor("dbg_hT", [128, 4, B], F32, kind="ExternalOutput")

    with (
        tile.TileContext(nc) as tc,
        tc.tile_pool(name="persist", bufs=1) as P,
        tc.tile_pool(name="dram", bufs=1, space="DRAM") as DR,
    ):
        # ====== constants / small weights ======
        cw = P.tile([81, 256], F32R, tag="cw")
        nc.gpsimd.dma_start(cw[:], d_cwT[:])
        cb0 = P.tile([128, 2], F32, tag="cb0")
        nc.sync.dma_start(cb0[:, 0:1], d_conv_b[0:128, :])
        nc.sync.dma_start(cb0[:, 1:2], d_conv_b[128:256, :])
        pb0 = P.tile([128, 2], F32, tag="pb0")
        nc.sync.dma_start(pb0[:, 0:1], d_pc_b[0:128, :])
        nc.sync.dma_start(pb0[:, 1:2], d_pc_b[128:256, :])
        ident = P.tile([128, 128], F32, tag="ident")
        nc.sync.dma_start(ident[:], d_ident[:])
        S2 = P.tile([128, 16], F32R, tag="S2")
        nc.gpsimd.dma_start(S2[:], d_S2[:])

        x2s = [P.tile([128, 6, B, 6], F32, tag=f"x2s{mt}", name=f"x2s{mt}") for mt in range(2)]

        # ================= conv phase (per batch half) =================
        with (
            tc.tile_pool(name="convsb", bufs=1) as CB,
            tc.tile_pool(name="imcpool", bufs=3) as IMC,
            tc.tile_pool(name="wstream", bufs=4) as WS,
            tc.tile_pool(name="ps_c1", bufs=2, space="PSUM") as PC1,
            tc.tile_pool(name="ps_c2", bufs=1, space="PSUM") as PC2,
        ):
            xact = [CB.tile([128, 20, BH, 20], F32R, tag=f"xact{mt}", name=f"xact{mt}")
                    for mt in range(2)]
            for bh in range(2):
                bsl = slice(bh * BH, (bh + 1) * BH)
                # ---- conv1 ----
                for y in range(20):
                    imc = IMC.tile([81, BH, 20], F32R, tag="imc")
                    nc.gpsimd.dma_start(imc[:], d_im2col[:, y, bsl, :])
                    for mt in range(2):
                        ps = PC1.tile([128, BH * 20], F32, tag="c1ps")
                        for n0, n1 in ((0, 384), (384, 640)):
                            nc.tensor.matmul(
                                ps[:, n0:n1],
                                cw[:, mt * 128:(mt + 1) * 128],
                                imc[:].rearrange("p a b -> p (a b)")[:, n0:n1],
                                start=True, stop=True,
                            )
                        dst = xact[mt][:, y, :, :].rearrange("p a b -> p (a b)")
                        if mt == 0:
                            nc.vector.tensor_scalar(
                                dst, ps[:], cb0[:, 0:1], 0.0, ALU.add, ALU.max)
                        else:
                            nc.scalar.activation(
                                dst, ps[:], AF.Relu, bias=cb0[:, 1:2], scale=1.0)
                if DEBUG and bh == 1:
                    for mt in range(2):
                        nc.sync.dma_start(dbg["xact"][mt], xact[mt][:].bitcast(F32))

                # ---- conv2 ----
                for mt in range(2):
                    ps2 = [PC2.tile([128, 2, BH, 6], F32, tag=f"c2ps{j}", name=f"c2ps{j}")
                           for j in range(3)]
                    for dydx in range(81):
                        dy, dx = divmod(dydx, 9)
                        wt = WS.tile([128, 2, 128], F32R, tag="wt")
                        nc.gpsimd.dma_start(wt[:], d_pcwT[mt, dydx])
                        for kh in range(2):
                            for oyp in range(3):
                                # rows 4*oyp + 2j + dy, j in {0,1}
                                rhs = xact[kh][:, 4 * oyp + dy:4 * oyp + dy + 3:2,
                                               :, dx:dx + 11:2]
                                nc.tensor.matmul(
                                    ps2[oyp][:], wt[:, kh, :], rhs,
                                    start=(dydx == 0 and kh == 0),
                                    stop=(dydx == 80 and kh == 1),
                                )
                    for oyp in range(3):
                        nc.vector.tensor_scalar(
                            x2s[mt][:, 2 * oyp:2 * oyp + 2, bsl, :],
                            ps2[oyp][:], pb0[:, mt:mt + 1], None, ALU.add,
                        )
        if DEBUG:
            for mt in range(2):
                nc.sync.dma_start(dbg["x2s"][mt], x2s[mt][:])

        # ================= u phase =================
        with tc.tile_pool(name="upool", bufs=1) as UP:
            u_byB = UP.tile([B, RI], F32R, tag="u_byB")
            uT = UP.tile([128, KT, B], F32R, tag="uT")

            with (
                tc.tile_pool(name="usc", bufs=1) as USC,
                tc.tile_pool(name="usc2", bufs=2) as USC2,
                tc.tile_pool(name="ps_tr", bufs=4, space="PSUM") as PTR,
            ):
                u_pre = USC.tile([B, RI], F32, tag="u_pre")
                for mt in range(2):
                    for oy in range(6):
                        for ox in range(6):
                            tp = PTR.tile([128, 128], F32, tag="tr")
                            nc.tensor.transpose(
                                tp[0:B, :], x2s[mt][:, oy, :, ox], ident[:])
                            dst = u_pre[:, mt * 4608 + oy * 6 + ox::36][:, 0:128]
                            nc.vector.tensor_copy(dst, tp[0:B, :])

                sn = USC.tile([B, R], F32, tag="sn")
                for ch in range(9):
                    sl = slice(ch * 1024, (ch + 1) * 1024)
                    sq = USC2.tile([B, 1024], F32, tag="sq")
                    nc.scalar.activation(sq[:], u_pre[:, sl], AF.Square)
                    nc.vector.tensor_reduce(
                        sn[:, ch * 128:(ch + 1) * 128],
                        sq[:].rearrange("p (g i) -> p g i", i=8),
                        AXL.X, ALU.add,
                    )
                fac = USC.tile([B, R], F32, tag="fac")
                rt = USC.tile([B, R], F32, tag="rt")
                nc.scalar.activation(rt[:], sn[:], AF.Sqrt)
                snp = USC.tile([B, R], F32, tag="snp")
                nc.vector.tensor_scalar(snp[:], sn[:], 1.0, None, ALU.add)
                rsnp = USC.tile([B, R], F32, tag="rsnp")
                nc.vector.reciprocal(rsnp[:], snp[:])
                nc.vector.tensor_tensor(fac[:], rt[:], rsnp[:], ALU.mult)
                nc.vector.tensor_tensor(
                    u_byB[:].rearrange("p (r i) -> p r i", i=8),
                    u_pre[:].rearrange("p (r i) -> p r i", i=8),
                    fac[:].unsqueeze(2).to_broadcast([B, R, 8]),
                    ALU.mult,
                )
                if DEBUG:
                    nc.sync.dma_start(dbg["u"][:], u_byB[:].bitcast(F32))
                for k in range(KT):
                    tp = PTR.tile([128, 128], F32, tag="tr")
                    nc.tensor.transpose(
                        tp[:, 0:B], u_byB[:, k * 128:(k + 1) * 128].bitcast(F32),
                        ident[0:B, 0:B])
                    nc.vector.tensor_copy(uT[:, k, :], tp[:, 0:B])

            # ================= routing phase =================
            vT0 = P.tile([128, B], F32R, tag="vT0")
            vT1 = P.tile([32, B], F32R, tag="vT1")
            with (
                tc.tile_pool(name="rt_big", bufs=1) as RB,
                tc.tile_pool(name="rt_sc", bufs=1) as RS,
                tc.tile_pool(name="rt_sc2", bufs=2) as RS2,
                tc.tile_pool(name="ps_st", bufs=1, space="PSUM") as PST,
                tc.tile_pool(name="ps_m", bufs=2, space="PSUM") as PSM,
                tc.tile_pool(name="ps_db", bufs=1, space="PSUM") as PDB,
                tc.tile_pool(name="ps_tp", bufs=1, space="PSUM") as PTP,
            ):
                Wmm = RB.tile([128, KT, CO], F32R, tag="Wmm")
                nc.gpsimd.dma_start(
                    Wmm[:], d_Wmm[:].rearrange("(k p) c -> p k c", p=128))
                W1mm = RB.tile([128, KT, CO], F32R, tag="W1mm")
                c_rep = RB.tile([128, KT, C], F32, tag="c_rep")
                bacc_t = RB.tile([C, R], F32, tag="bacc")
                c_T = RB.tile([C, R], F32, tag="c_T")
                c_byR = RB.tile([128, 9, C], F32, tag="c_byR")

                d_crdram = DR.tile([R, C], F32)
                d_red_in = DR.tile([C, R], F32)
                d_red_out = DR.tile([C, R], F32)

                for it in range(NUM_ITERS):
                    # ---- s_t = W'^T @ u ----
                    st0 = PST.tile([128, B], F32, tag="st0")
                    st1 = PST.tile([32, B], F32, tag="st1")
                    lhs = Wmm if it == 0 else W1mm
                    for k in range(KT):
                        nc.tensor.matmul(st0[:], lhs[:, k, 0:128], uT[:, k, :],
                                         start=(k == 0), stop=(k == KT - 1))
                    for k in range(KT):
                        nc.tensor.matmul(st1[:], lhs[:, k, 128:160], uT[:, k, :],
                                         start=(k == 0), stop=(k == KT - 1))
                    # ---- v = squash(s) elementwise ----
                    for half, (st, vt, np_) in enumerate(
                            ((st0, vT0, 128), (st1, vT1, 32))):
                        s_sb = RS2.tile([np_, B], F32, tag=f"s_sb{half}")
                        if it == 0:
                            nc.vector.tensor_scalar(
                                s_sb[:], st[:], 1.0 / R, None, ALU.mult)
                        else:
                            nc.vector.tensor_copy(s_sb[:], st[:])
                        if DEBUG and it == 0 and half == 0:
                            nc.sync.dma_start(dbg["s0T"][:], s_sb[:])
                        t2 = RS2.tile([np_, B], F32, tag=f"t2_{half}")
                        nc.vector.tensor_tensor(t2[:], s_sb[:], s_sb[:], ALU.mult)
                        num = RS2.tile([np_, B], F32, tag=f"num{half}")
                        nc.vector.tensor_tensor(num[:], t2[:], s_sb[:], ALU.mult)
                        rte = RS2.tile([np_, B], F32, tag=f"rte{half}")
                        nc.scalar.activation(rte[:], t2[:], AF.Sqrt)
                        den = RS2.tile([np_, B], F32, tag=f"den{half}")
                        nc.vector.tensor_scalar(den[:], t2[:], 1.0, None, ALU.add)
                        nc.vector.tensor_tensor(den[:], den[:], rte[:], ALU.mult)
                        rden = RS2.tile([np_, B], F32, tag=f"rden{half}")
                        nc.vector.reciprocal(rden[:], den[:])
                        nc.vector.tensor_tensor(vt[:], num[:], rden[:], ALU.mult)

                    if it == NUM_ITERS - 1:
                        break

                    # ---- v -> [B, CO] ----
                    v_b = RS.tile([B, CO], F32R, tag="v_b")
                    tpv0 = PTP.tile([128, 128], F32, tag="tp")
                    nc.tensor.transpose(tpv0[0:B, :], vT0[:].bitcast(F32), ident[:])
                    nc.vector.tensor_copy(v_b[:, 0:128], tpv0[0:B, :])
                    tpv1 = PTP.tile([128, 128], F32, tag="tp")
                    nc.tensor.transpose(tpv1[0:B, 0:32], vT1[:].bitcast(F32), ident[0:32, 0:32])
                    nc.vector.tensor_copy(v_b[:, 128:160], tpv1[0:B, 0:32])

                    # ---- db^T[c, r] = sum_oi W*(u^T v)/Btot via per-ktile chain ----
                    db_ps = PDB.tile([C, R], F32, tag="db_ps")
                    for k in range(KT):
                        Mps = PSM.tile([128, CO], F32, tag="Mps")
                        nc.tensor.matmul(
                            Mps[:], u_byB[:, k * 128:(k + 1) * 128], v_b[:],
                            start=True, stop=True)
                        prod = RS2.tile([128, CO], F32, tag="prod")
                        nc.vector.tensor_tensor(
                            prod[:], Mps[:], Wmm[:, k, :], ALU.mult)
                        Tk = RS2.tile([128, C], F32R, tag="Tk")
                        with nc.allow_low_precision(reason="f32r rounding for PE"):
                            nc.vector.tensor_reduce(
                                Tk[:], prod[:].rearrange("p (c o) -> p c o", o=O),
                                AXL.X, ALU.add)
                        nc.tensor.matmul(
                            db_ps[:, k * 16:(k + 1) * 16], Tk[:], S2[:],
                            start=True, stop=True)
                    db_sb = RS.tile([C, R], F32, tag="db_sb")
                    nc.vector.tensor_copy(db_sb[:], db_ps[:])

                    # ---- AllReduce batch-mean across cores ----
                    nc.sync.dma_start(d_red_in[:], db_sb[:])
                    nc.gpsimd.collective_compute(
                        "AllReduce", ALU.add,
                        replica_groups=[list(range(N_CORES))],
                        ins=[d_red_in.opt()], outs=[d_red_out.opt()],
                    )
                    db_red = RS.tile([C, R], F32, tag="db_red")
                    nc.sync.dma_start(db_red[:], d_red_out[:])
                    if it == 0:
                        nc.vector.tensor_copy(bacc_t[:], db_red[:])
                        if DEBUG:
                            nc.sync.dma_start(dbg["db0"][:], bacc_t[:])
                    else:
                        nc.vector.tensor_tensor(
                            bacc_t[:], bacc_t[:], db_red[:], ALU.add)

                    # ---- c = softmax_r(b) on [C, R] ----
                    mx = RS.tile([C, 1], F32, tag="mx")
                    nc.vector.tensor_reduce(mx[:], bacc_t[:], AXL.X, ALU.max)
                    nmx = RS.tile([C, 1], F32, tag="nmx")
                    nc.vector.tensor_scalar(nmx[:], mx[:], -1.0, None, ALU.mult)
                    ex = RS.tile([C, R], F32, tag="ex")
                    nc.scalar.activation(ex[:], bacc_t[:], AF.Exp,
                                         bias=nmx[:], scale=1.0)
                    sm = RS.tile([C, 1], F32, tag="sm")
                    nc.vector.tensor_reduce(sm[:], ex[:], AXL.X, ALU.add)
                    rcp = RS.tile([C, 1], F32, tag="rcp")
                    nc.vector.reciprocal(rcp[:], sm[:])
                    nc.vector.tensor_scalar(c_T[:], ex[:], rcp[:], None, ALU.mult)
                    if DEBUG and it == 0:
                        nc.sync.dma_start(dbg["c1T"][:], c_T[:])

                    # ---- c_rep[(r16,i),(k,c)] via DRAM round trip ----
                    for rb in range(9):
                        tpc = PTP.tile([128, 128], F32, tag="tp")
                        nc.tensor.transpose(
                            tpc[:, 0:C], c_T[:, rb * 128:(rb + 1) * 128],
                            ident[0:C, 0:C])
                        nc.vector.tensor_copy(c_byR[:, rb, :], tpc[:, 0:C])
                    nc.sync.dma_start(
                        d_crdram[:].rearrange("(a p) c -> p a c", p=128),
                        c_byR[:])
                    for rlo in range(16):
                        nc.sync.dma_start(
                            c_rep[rlo * 8:(rlo + 1) * 8, :, :],
                            d_crdram[:].rearrange("(k rl) c -> rl k c", rl=16)[rlo]
                            .unsqueeze(0).to_broadcast([8, KT, C]),
                        )
                    # ---- W' = W * c (broadcast over o) ----
                    nc.vector.tensor_tensor(
                        W1mm[:].rearrange("p k (c o) -> p k c o", o=O),
                        Wmm[:].rearrange("p k (c o) -> p k c o", o=O),
                        c_rep[:].unsqueeze(3).to_broadcast([128, KT, C, O]),
                        ALU.mult,
                    )

        # ================= encoder =================
        if DEBUG:
            nc.sync.dma_start(dbg["vT2"][0:128, :], vT0[:].bitcast(F32))
            nc.sync.dma_start(dbg["vT2"][128:160, :], vT1[:].bitcast(F32))
        with (
            tc.tile_pool(name="enc", bufs=1) as EN,
            tc.tile_pool(name="enc2", bufs=2) as EN2,
            tc.tile_pool(name="ps_h", bufs=2, space="PSUM") as PSH,
            tc.tile_pool(name="ps_o", bufs=1, space="PSUM") as PSO,
        ):
            w1T0 = EN.tile([128, 512], F32R, tag="w1T0")
            nc.gpsimd.dma_start(w1T0[:], d_w1T[0:128, :])
            w1T1 = EN.tile([32, 512], F32R, tag="w1T1")
            nc.gpsimd.dma_start(w1T1[:], d_w1T[128:160, :])
            b1sb = EN.tile([128, 4], F32, tag="b1sb")
            for jt in range(4):
                nc.sync.dma_start(b1sb[:, jt:jt + 1],
                                  d_b1[jt * 128:(jt + 1) * 128, :])
            w2T = EN.tile([128, 4, 10], F32R, tag="w2T")
            nc.gpsimd.dma_start(
                w2T[:], d_w2T[:].rearrange("(j p) c -> p j c", p=128))
            b2sb = EN.tile([B, 9], F32, tag="b2sb")
            nc.sync.dma_start(b2sb[:], d_b2rep[:])

            hT = EN.tile([128, 4, B], F32R, tag="hT")
            for jt in range(4):
                hp = PSH.tile([128, B], F32, tag="hp")
                nc.tensor.matmul(hp[:], w1T0[:, jt * 128:(jt + 1) * 128], vT0[:],
                                 start=True, stop=False)
                nc.tensor.matmul(hp[:], w1T1[:, jt * 128:(jt + 1) * 128], vT1[:],
                                 start=False, stop=True)
                nc.scalar.activation(hT[:, jt, :], hp[:], AF.Relu,
                                     bias=b1sb[:, jt:jt + 1], scale=1.0)
            if DEBUG:
                nc.sync.dma_start(dbg["hT"][:], hT[:].bitcast(F32))

            op = PSO.tile([B, 10], F32, tag="op")
            for jt in range(4):
                nc.tensor.matmul(op[:], hT[:, jt, :], w2T[:, jt, :],
                                 start=(jt == 0), stop=(jt == 3))
            logit = EN2.tile([B, 9], F32, tag="logit")
            nc.vector.tensor_tensor(logit[:], op[:, 0:9], b2sb[:], ALU.add)
            out_sb = EN2.tile([B, 9], F32, tag="out_sb")
            nc.scalar.activation(out_sb[:], logit[:], AF.Sigmoid)
            nc.sync.dma_start(d_out[:], out_sb[:])

    nc.compile()
    return nc


def _host_prep(data, conv_w, conv_b, pc_w, pc_b, W, enc_w1, enc_b1, enc_w2, enc_b2):
    """Layout-only host prep. Returns (shared_inputs, per_core_im2col)."""
    Bfull = data.shape[0]
    assert Bfull == N_CORES * B
    d = np.ascontiguousarray(data[:, 0])  # [512, 28, 28]
    sw = np.lib.stride_tricks.sliding_window_view(d, (9, 9), axis=(1, 2))
    # sw: [Bfull, 20, 20, 9, 9] -> (dy,dx,y,b,x)
    im2col_all = np.ascontiguousarray(sw.transpose(3, 4, 1, 0, 2)).reshape(
        81, 20, Bfull, 20)
    per_core = [np.ascontiguousarray(im2col_all[:, :, c * B:(c + 1) * B, :])
                for c in range(N_CORES)]

    cwT = np.ascontiguousarray(conv_w.reshape(256, 81).T)          # [81, 256]
    pcwT = np.ascontiguousarray(
        pc_w.reshape(2, 128, 2, 128, 81).transpose(0, 4, 3, 2, 1))  # [2,81,128,2,128]
    Wmm = np.ascontiguousarray(W.transpose(0, 3, 1, 2).reshape(RI, CO))
    S2 = np.zeros((128, 16), dtype=np.float32)
    for rlo in range(16):
        for i in range(8):
            S2[rlo * 8 + i, rlo] = 1.0 / (N_CORES * B)
    shared = dict(
        cwT=cwT,
        conv_b=np.asarray(conv_b, np.float32).reshape(256, 1),
        pcwT=pcwT,
        pc_b=np.asarray(pc_b, np.float32).reshape(256, 1),
        Wmm=Wmm,
        S2=S2,
        ident=np.eye(128, dtype=np.float32),
        w1T=np.ascontiguousarray(np.asarray(enc_w1, np.float32).T),   # [160, 512]
        b1=np.asarray(enc_b1, np.float32).reshape(512, 1),
        w2T=np.ascontiguousarray(np.pad(np.asarray(enc_w2, np.float32).T,
                                        ((0, 0), (0, 1)))),          # [512, 10]
        b2rep=np.tile(np.asarray(enc_b2, np.float32).reshape(1, 9), (B, 1)),
    )
    return shared, per_core


def kernel(**inputs):
    global _CACHED_NC
    if _CACHED_NC is None:
        _CACHED_NC = build()
    nc = _CACHED_NC
    inputs = {k: np.asarray(v, dtype=np.float32) for k, v in inputs.items()}
    shared, per_core = _host_prep(**inputs)
    in_maps = [dict(shared, im2col=per_core[c]) for c in range(N_CORES)]
    res = bass_utils.run_bass_kernel_spmd(nc, in_maps, core_ids=list(range(N_CORES)))
    out = np.concatenate([res.results[c]["out"] for c in range(N_CORES)], axis=0)
    return out


if __name__ == "__main__":
    import reference
    inputs = {k: np.asarray(v) for k, v in reference.setup_inputs().items()}
    got = kernel(**inputs)
    exp = np.asarray(reference.reference(**inputs))
    rel = np.abs(got - exp).max() / np.abs(exp).max()
    print("Relative error:", rel)


# revision 12
# speedup vs baseline: 1.0046x; 1.0028x over previous
"""CapsNet forward pass on 8 Trainium2 NeuronCores (Bass/Tile).

Data-parallel: batch 512 sharded 64/core; parameters replicated; the routing
b_ij batch-mean update is an AllReduce. Host-side prep is layout-only
(im2col of the input, weight transposes) - all FLOPs run on device.

Self-contained: hardcodes all shapes from the problem spec.
"""
import os
import numpy as np

import concourse.bacc as bacc
import concourse.tile as tile
from concourse import bass_utils, mybir

F32 = mybir.dt.float32
F32R = mybir.dt.float32r
AF = mybir.ActivationFunctionType
ALU = mybir.AluOpType
AXL = mybir.AxisListType

N_CORES = 8
B = 64            # batch per core
BH = 32           # batch per conv sub-pass
R = 1152          # num routes
C = 10            # num capsules
O = 16            # out dim
RI = R * 8        # 9216
CO = C * O        # 160
KT = RI // 128    # 72 k-tiles over (r,i)
NUM_ITERS = 3

DEBUG = bool(int(os.environ.get("BASS_CAPS_DEBUG", "0")))

_CACHED_NC = None


def build():
    nc = bacc.Bacc("TRN2", target_bir_lowering=False, debug=False,
                   num_devices=N_CORES)

    # ---------------- DRAM I/O ----------------
    d_im2col = nc.dram_tensor("im2col", [81, 20, B, 20], F32, kind="ExternalInput")
    d_cwT = nc.dram_tensor("cwT", [81, 256], F32, kind="ExternalInput")
    d_conv_b = nc.dram_tensor("conv_b", [256, 1], F32, kind="ExternalInput")
    d_pcwT = nc.dram_tensor("pcwT", [2, 81, 128, 2, 128], F32, kind="ExternalInput")
    d_pc_b = nc.dram_tensor("pc_b", [256, 1], F32, kind="ExternalInput")
    d_Wmm = nc.dram_tensor("Wmm", [RI, CO], F32, kind="ExternalInput")
    d_S2 = nc.dram_tensor("S2", [128, 16], F32, kind="ExternalInput")
    d_ident = nc.dram_tensor("ident", [128, 128], F32, kind="ExternalInput")
    d_w1T = nc.dram_tensor("w1T", [CO, 512], F32, kind="ExternalInput")
    d_b1 = nc.dram_tensor("b1", [512, 1], F32, kind="ExternalInput")
    d_w2T = nc.dram_tensor("w2T", [512, 10], F32, kind="ExternalInput")
    d_b2rep = nc.dram_tensor("b2rep", [B, 9], F32, kind="ExternalInput")
    d_out = nc.dram_tensor("out", [B, 9], F32, kind="ExternalOutput")

    dbg = {}
    if DEBUG:
        dbg["xact"] = nc.dram_tensor("dbg_xact", [2, 128, 20, BH, 20], F32, kind="ExternalOutput")
        dbg["x2s"] = nc.dram_tensor("dbg_x2s", [2, 128, 6, B, 6], F32, kind="ExternalOutput")
        dbg["u"] = nc.dram_tensor("dbg_u", [B, RI], F32, kind="ExternalOutput")
        dbg["s0T"] = nc.dram_tensor("dbg_s0T", [128, B], F32, kind="ExternalOutput")
        dbg["db0"] = nc.dram_tensor("dbg_db0", [C, R], F32, kind="ExternalOutput")
        dbg["c1T"] = nc.dram_tensor("dbg_c1T", [C, R], F32, kind="ExternalOutput")
        dbg["vT2"] = nc.dram_tensor("dbg_vT2", [CO, B], F32, kind="ExternalOutput")
        dbg["hT"] = nc.dram_tensor("dbg_hT", [128, 4, B], F32, kind="ExternalOutput")

    with (
        tile.TileContext(nc) as tc,
        tc.tile_pool(name="persist", bufs=1) as P,
        tc.tile_pool(name="dram", bufs=1, space="DRAM") as DR,
    ):
        # ====== constants / small weights ======
        cw = P.tile([81, 256], F32R, tag="cw")
        nc.gpsimd.dma_start(cw[:], d_cwT[:])
        cb0 = P.tile([128, 2], F32, tag="cb0")
        nc.sync.dma_start(cb0[:, 0:1], d_conv_b[0:128, :])
        nc.sync.dma_start(cb0[:, 1:2], d_conv_b[128:256, :])
        pb0 = P.tile([128, 2], F32, tag="pb0")
        nc.sync.dma_start(pb0[:, 0:1], d_pc_b[0:128, :])
        nc.sync.dma_start(pb0[:, 1:2], d_pc_b[128:256, :])
        ident = P.tile([128, 128], F32, tag="ident")
        nc.sync.dma_start(ident[:], d_ident[:])
        S2 = P.tile([128, 16], F32R, tag="S2")
        nc.gpsimd.dma_start(S2[:], d_S2[:])

        x2s = [P.tile([128, 6, B, 6], F32, tag=f"x2s{mt}", name=f"x2s{mt}") for mt in range(2)]

        # ================= conv phase (per batch half) =================
        with (
            tc.tile_pool(name="convsb", bufs=1) as CB,
            tc.tile_pool(name="imcpool", bufs=3) as IMC,
            tc.tile_pool(name="wstream", bufs=4) as WS,
            tc.tile_pool(name="ps_c1", bufs=2, space="PSUM") as PC1,
            tc.tile_pool(name="ps_c2", bufs=1, space="PSUM") as PC2,
        ):
            xact = [CB.tile([128, 20, BH, 20], F32R, tag=f"xact{mt}", name=f"xact{mt}")
                    for mt in range(2)]
            for bh in range(2):
                bsl = slice(bh * BH, (bh + 1) * BH)
                # ---- conv1 ----
                for y in range(20):
                    imc = IMC.tile([81, BH, 20], F32R, tag="imc")
                    nc.gpsimd.dma_start(imc[:], d_im2col[:, y, bsl, :])
                    for mt in range(2):
                        ps = PC1.tile([128, BH * 20], F32, tag="c1ps")
                        for n0, n1 in ((0, 512), (512, 640)):
                            nc.tensor.matmul(
                                ps[:, n0:n1],
                                cw[:, mt * 128:(mt + 1) * 128],
                                imc[:].rearrange("p a b -> p (a b)")[:, n0:n1],
                                start=True, stop=True,
                            )
                        dst = xact[mt][:, y, :, :].rearrange("p a b -> p (a b)")
                        if mt == 0:
                            nc.vector.tensor_scalar(
                                dst, ps[:], cb0[:, 0:1], 0.0, ALU.add, ALU.max)
                        else:
                            nc.scalar.activation(
                                dst, ps[:], AF.Relu, bias=cb0[:, 1:2], scale=1.0)
                if DEBUG and bh == 1:
                    for mt in range(2):
                        nc.sync.dma_start(dbg["xact"][mt], xact[mt][:].bitcast(F32))

                # ---- conv2 ----
                for mt in range(2):
                    ps2 = [PC2.tile([128, 2, BH, 6], F32, tag=f"c2ps{j}", name=f"c2ps{j}")
                           for j in range(3)]
                    for dydx in range(81):
                        dy, dx = divmod(dydx, 9)
                        wt = WS.tile([128, 2, 128], F32R, tag="wt")
                        nc.gpsimd.dma_start(wt[:], d_pcwT[mt, dydx])
                        for kh in range(2):
                            for oyp in range(3):
                                # rows 4*oyp + 2j + dy, j in {0,1}
                                rhs = xact[kh][:, 4 * oyp + dy:4 * oyp + dy + 3:2,
                                               :, dx:dx + 11:2]
                                nc.tensor.matmul(
                                    ps2[oyp][:], wt[:, kh, :], rhs,
                                    start=(dydx == 0 and kh == 0),
                                    stop=(dydx == 80 and kh == 1),
                                )
                    for oyp in range(3):
                        nc.vector.tensor_scalar(
                            x2s[mt][:, 2 * oyp:2 * oyp + 2, bsl, :],
                            ps2[oyp][:], pb0[:, mt:mt + 1], None, ALU.add,
                        )
        if DEBUG:
            for mt in range(2):
                nc.sync.dma_start(dbg["x2s"][mt], x2s[mt][:])

        # ================= u phase =================
        with tc.tile_pool(name="upool", bufs=1) as UP:
            u_byB = UP.tile([B, RI], F32R, tag="u_byB")
            uT = UP.tile([128, KT, B], F32R, tag="uT")

            with (
                tc.tile_pool(name="usc", bufs=1) as USC,
                tc.tile_pool(name="usc2", bufs=2) as USC2,
                tc.tile_pool(name="ps_tr", bufs=4, space="PSUM") as PTR,
            ):
                u_pre = USC.tile([B, RI], F32, tag="u_pre")
                for mt in range(2):
                    for oy in range(6):
                        for ox in range(6):
                            tp = PTR.tile([128, 128], F32, tag="tr")
                            nc.tensor.transpose(
                                tp[0:B, :], x2s[mt][:, oy, :, ox], ident[:])
                            dst = u_pre[:, mt * 4608 + oy * 6 + ox::36][:, 0:128]
                            nc.vector.tensor_copy(dst, tp[0:B, :])

                sn = USC.tile([B, R], F32, tag="sn")
                for ch in range(9):
                    sl = slice(ch * 1024, (ch + 1) * 1024)
                    sq = USC2.tile([B, 1024], F32, tag="sq")
                    nc.scalar.activation(sq[:], u_pre[:, sl], AF.Square)
                    nc.vector.tensor_reduce(
                        sn[:, ch * 128:(ch + 1) * 128],
                        sq[:].rearrange("p (g i) -> p g i", i=8),
                        AXL.X, ALU.add,
                    )
                fac = USC.tile([B, R], F32, tag="fac")
                rt = USC.tile([B, R], F32, tag="rt")
                nc.scalar.activation(rt[:], sn[:], AF.Sqrt)
                snp = USC.tile([B, R], F32, tag="snp")
                nc.vector.tensor_scalar(snp[:], sn[:], 1.0, None, ALU.add)
                rsnp = USC.tile([B, R], F32, tag="rsnp")
                nc.vector.reciprocal(rsnp[:], snp[:])
                nc.vector.tensor_tensor(fac[:], rt[:], rsnp[:], ALU.mult)
                nc.vector.tensor_tensor(
                    u_byB[:].rearrange("p (r i) -> p r i", i=8),
                    u_pre[:].rearrange("p (r i) -> p r i", i=8),
                    fac[:].unsqueeze(2).to_broadcast([B, R, 8]),
                    ALU.mult,
                )
                if DEBUG:
                    nc.sync.dma_start(dbg["u"][:], u_byB[:].bitcast(F32))
                for k in range(KT):
                    tp = PTR.tile([128, 128], F32, tag="tr")
                    nc.tensor.transpose(
                        tp[:, 0:B], u_byB[:, k * 128:(k + 1) * 128].bitcast(F32),
                        ident[0:B, 0:B])
                    nc.vector.tensor_copy(uT[:, k, :], tp[:, 0:B])

            # ================= routing phase =================
            vT0 = P.tile([128, B], F32R, tag="vT0")
            vT1 = P.tile([32, B], F32R, tag="vT1")
            with (
                tc.tile_pool(name="rt_big", bufs=1) as RB,
                tc.tile_pool(name="rt_sc", bufs=1) as RS,
                tc.tile_pool(name="rt_sc2", bufs=2) as RS2,
                tc.tile_pool(name="ps_st", bufs=1, space="PSUM") as PST,
                tc.tile_pool(name="ps_m", bufs=2, space="PSUM") as PSM,
                tc.tile_pool(name="ps_db", bufs=1, space="PSUM") as PDB,
                tc.tile_pool(name="ps_tp", bufs=1, space="PSUM") as PTP,
            ):
                Wmm = RB.tile([128, KT, CO], F32R, tag="Wmm")
                nc.gpsimd.dma_start(
                    Wmm[:], d_Wmm[:].rearrange("(k p) c -> p k c", p=128))
                W1mm = RB.tile([128, KT, CO], F32R, tag="W1mm")
                c_rep = RB.tile([128, KT, C], F32, tag="c_rep")
                bacc_t = RB.tile([C, R], F32, tag="bacc")
                c_T = RB.tile([C, R], F32, tag="c_T")
                c_byR = RB.tile([128, 9, C], F32, tag="c_byR")

                d_crdram = DR.tile([R, C], F32)
                d_red_in = DR.tile([C, R], F32)
                d_red_out = DR.tile([C, R], F32)

                for it in range(NUM_ITERS):
                    # ---- s_t = W'^T @ u ----
                    st0 = PST.tile([128, B], F32, tag="st0")
                    st1 = PST.tile([32, B], F32, tag="st1")
                    lhs = Wmm if it == 0 else W1mm
                    for k in range(KT):
                        nc.tensor.matmul(st0[:], lhs[:, k, 0:128], uT[:, k, :],
                                         start=(k == 0), stop=(k == KT - 1))
                    for k in range(KT):
                        nc.tensor.matmul(st1[:], lhs[:, k, 128:160], uT[:, k, :],
                                         start=(k == 0), stop=(k == KT - 1))
                    # ---- v = squash(s) elementwise ----
                    for half, (st, vt, np_) in enumerate(
                            ((st0, vT0, 128), (st1, vT1, 32))):
                        s_sb = RS2.tile([np_, B], F32, tag=f"s_sb{half}")
                        if it == 0:
                            nc.vector.tensor_scalar(
                                s_sb[:], st[:], 1.0 / R, None, ALU.mult)
                        else:
                            nc.vector.tensor_copy(s_sb[:], st[:])
                        if DEBUG and it == 0 and half == 0:
                            nc.sync.dma_start(dbg["s0T"][:], s_sb[:])
                        t2 = RS2.tile([np_, B], F32, tag=f"t2_{half}")
                        nc.vector.tensor_tensor(t2[:], s_sb[:], s_sb[:], ALU.mult)
                        num = RS2.tile([np_, B], F32, tag=f"num{half}")
                        nc.vector.tensor_tensor(num[:], t2[:], s_sb[:], ALU.mult)
                        rte = RS2.tile([np_, B], F32, tag=f"rte{half}")
                        nc.scalar.activation(rte[:], t2[:], AF.Sqrt)
                        den = RS2.tile([np_, B], F32, tag=f"den{half}")
                        nc.vector.tensor_scalar(den[:], t2[:], 1.0, None, ALU.add)
                        nc.vector.tensor_tensor(den[:], den[:], rte[:], ALU.mult)
                        rden = RS2.tile([np_, B], F32, tag=f"rden{half}")
                        nc.vector.reciprocal(rden[:], den[:])
                        nc.vector.tensor_tensor(vt[:], num[:], rden[:], ALU.mult)

                    if it == NUM_ITERS - 1:
                        break

                    # ---- v -> [B, CO] ----
                    v_b = RS.tile([B, CO], F32R, tag="v_b")
                    tpv0 = PTP.tile([128, 128], F32, tag="tp")
                    nc.tensor.transpose(tpv0[0:B, :], vT0[:].bitcast(F32), ident[:])
                    nc.vector.tensor_copy(v_b[:, 0:128], tpv0[0:B, :])
                    tpv1 = PTP.tile([128, 128], F32, tag="tp")
                    nc.tensor.transpose(tpv1[0:B, 0:32], vT1[:].bitcast(F32), ident[0:32, 0:32])
                    nc.vector.tensor_copy(v_b[:, 128:160], tpv1[0:B, 0:32])

                    # ---- db^T[c, r] = sum_oi W*(u^T v)/Btot via per-ktile chain ----
                    db_ps = PDB.tile([C, R], F32, tag="db_ps")
                    for k in range(KT):
                        Mps = PSM.tile([128, CO], F32, tag="Mps")
                        nc.tensor.matmul(
                            Mps[:], u_byB[:, k * 128:(k + 1) * 128], v_b[:],
                            start=True, stop=True)
                        prod = RS2.tile([128, CO], F32, tag="prod")
                        nc.vector.tensor_tensor(
                            prod[:], Mps[:], Wmm[:, k, :], ALU.mult)
                        Tk = RS2.tile([128, C], F32R, tag="Tk")
                        with nc.allow_low_precision(reason="f32r rounding for PE"):
                            nc.vector.tensor_reduce(
                                Tk[:], prod[:].rearrange("p (c o) -> p c o", o=O),
                                AXL.X, ALU.add)
                        nc.tensor.matmul(
                            db_ps[:, k * 16:(k + 1) * 16], Tk[:], S2[:],
                            start=True, stop=True)
                    db_sb = RS.tile([C, R], F32, tag="db_sb")
                    nc.vector.tensor_copy(db_sb[:], db_ps[:])

                    # ---- AllReduce batch-mean across cores ----
                    nc.sync.dma_start(d_red_in[:], db_sb[:])
                    nc.gpsimd.collective_compute(
                        "AllReduce", ALU.add,
                        replica_groups=[list(range(N_CORES))],
                        ins=[d_red_in.opt()], outs=[d_red_out.opt()],
                    )
                    db_red = RS.tile([C, R], F32, tag="db_red")
                    nc.sync.dma_start(db_red[:], d_red_out[:])
                    if it == 0:
                        nc.vector.tensor_copy(bacc_t[:], db_red[:])
                        if DEBUG:
                            nc.sync.dma_start(dbg["db0"][:], bacc_t[:])
                    else:
                        nc.vector.tensor_tensor(
                            bacc_t[:], bacc_t[:], db_red[:], ALU.add)

                    # ---- c = softmax_r(b) on [C, R] ----
                    mx = RS.tile([C, 1], F32, tag="mx")
                    nc.vector.tensor_reduce(mx[:], bacc_t[:], AXL.X, ALU.max)
                    nmx = RS.tile([C, 1], F32, tag="nmx")
                    nc.vector.tensor_scalar(nmx[:], mx[:], -1.0, None, ALU.mult)
                    ex = RS.tile([C, R], F32, tag="ex")
                    nc.scalar.activation(ex[:], bacc_t[:], AF.Exp,
                                         bias=nmx[:], scale=1.0)
                    sm = RS.tile([C, 1], F32, tag="sm")
                    nc.vector.tensor_reduce(sm[:], ex[:], AXL.X, ALU.add)
                    rcp = RS.tile([C, 1], F32, tag="rcp")
                    nc.vector.reciprocal(rcp[:], sm[:])
                    nc.vector.tensor_scalar(c_T[:], ex[:], rcp[:], None, ALU.mult)
                    if DEBUG and it == 0:
                        nc.sync.dma_start(dbg["c1T"][:], c_T[:])

                    # ---- c_rep[(r16,i),(k,c)] via DRAM round trip ----
                    for rb in range(9):
                        tpc = PTP.tile([128, 128], F32, tag="tp")
                        nc.tensor.transpose(
                            tpc[:, 0:C], c_T[:, rb * 128:(rb + 1) * 128],
                            ident[0:C, 0:C])
                        nc.vector.tensor_copy(c_byR[:, rb, :], tpc[:, 0:C])
                    nc.sync.dma_start(
                        d_crdram[:].rearrange("(a p) c -> p a c", p=128),
                        c_byR[:])
                    for rlo in range(16):
                        nc.sync.dma_start(
                            c_rep[rlo * 8:(rlo + 1) * 8, :, :],
                            d_crdram[:].rearrange("(k rl) c -> rl k c", rl=16)[rlo]
                            .unsqueeze(0).to_broadcast([8, KT, C]),
                        )
                    # ---- W' = W * c (broadcast over o) ----
                    nc.vector.tensor_tensor(
                        W1mm[:].rearrange("p k (c o) -> p k c o", o=O),
                        Wmm[:].rearrange("p k (c o) -> p k c o", o=O),
                        c_rep[:].unsqueeze(3).to_broadcast([128, KT, C, O]),
                        ALU.mult,
                    )

        # ================= encoder =================
        if DEBUG:
            nc.sync.dma_start(dbg["vT2"][0:128, :], vT0[:].bitcast(F32))
            nc.sync.dma_start(dbg["vT2"][128:160, :], vT1[:].bitcast(F32))
        with (
            tc.tile_pool(name="enc", bufs=1) as EN,
            tc.tile_pool(name="enc2", bufs=2) as EN2,
            tc.tile_pool(name="ps_h", bufs=2, space="PSUM") as PSH,
            tc.tile_pool(name="ps_o", bufs=1, space="PSUM") as PSO,
        ):
            w1T0 = EN.tile([128, 512], F32R, tag="w1T0")
            nc.gpsimd.dma_start(w1T0[:], d_w1T[0:128, :])
            w1T1 = EN.tile([32, 512], F32R, tag="w1T1")
            nc.gpsimd.dma_start(w1T1[:], d_w1T[128:160, :])
            b1sb = EN.tile([128, 4], F32, tag="b1sb")
            for jt in range(4):
                nc.sync.dma_start(b1sb[:, jt:jt + 1],
                                  d_b1[jt * 128:(jt + 1) * 128, :])
            w2T = EN.tile([128, 4, 10], F32R, tag="w2T")
            nc.gpsimd.dma_start(
                w2T[:], d_w2T[:].rearrange("(j p) c -> p j c", p=128))
            b2sb = EN.tile([B, 9], F32, tag="b2sb")
            nc.sync.dma_start(b2sb[:], d_b2rep[:])

            hT = EN.tile([128, 4, B], F32R, tag="hT")
            for jt in range(4):
                hp = PSH.tile([128, B], F32, tag="hp")
                nc.tensor.matmul(hp[:], w1T0[:, jt * 128:(jt + 1) * 128], vT0[:],
                                 start=True, stop=False)
                nc.tensor.matmul(hp[:], w1T1[:, jt * 128:(jt + 1) * 128], vT1[:],
                                 start=False, stop=True)
                nc.scalar.activation(hT[:, jt, :], hp[:], AF.Relu,
                                     bias=b1sb[:, jt:jt + 1], scale=1.0)
            if DEBUG:
                nc.sync.dma_start(dbg["hT"][:], hT[:].bitcast(F32))

            op = PSO.tile([B, 10], F32, tag="op")
            for jt in range(4):
                nc.tensor.matmul(op[:], hT[:, jt, :], w2T[:, jt, :],
                                 start=(jt == 0), stop=(jt == 3))
            logit = EN2.tile([B, 9], F32, tag="logit")
            nc.vector.tensor_tensor(logit[:], op[:, 0:9], b2sb[:], ALU.add)
            out_sb = EN2.tile([B, 9], F32, tag="out_sb")
            nc.scalar.activation(out_sb[:], logit[:], AF.Sigmoid)
            nc.sync.dma_start(d_out[:], out_sb[:])

    nc.compile()
    return nc


def _host_prep(data, conv_w, conv_b, pc_w, pc_b, W, enc_w1, enc_b1, enc_w2, enc_b2):
    """Layout-only host prep. Returns (shared_inputs, per_core_im2col)."""
    Bfull = data.shape[0]
    assert Bfull == N_CORES * B
    d = np.ascontiguousarray(data[:, 0])  # [512, 28, 28]
    sw = np.lib.stride_tricks.sliding_window_view(d, (9, 9), axis=(1, 2))
    # sw: [Bfull, 20, 20, 9, 9] -> (dy,dx,y,b,x)
    im2col_all = np.ascontiguousarray(sw.transpose(3, 4, 1, 0, 2)).reshape(
        81, 20, Bfull, 20)
    per_core = [np.ascontiguousarray(im2col_all[:, :, c * B:(c + 1) * B, :])
                for c in range(N_CORES)]

    cwT = np.ascontiguousarray(conv_w.reshape(256, 81).T)          # [81, 256]
    pcwT = np.ascontiguousarray(
        pc_w.reshape(2, 128, 2, 128, 81).transpose(0, 4, 3, 2, 1))  # [2,81,128,2,128]
    Wmm = np.ascontiguousarray(W.transpose(0, 3, 1, 2).reshape(RI, CO))
    S2 = np.zeros((128, 16), dtype=np.float32)
    for rlo in range(16):
        for i in range(8):
            S2[rlo * 8 + i, rlo] = 1.0 / (N_CORES * B)
    shared = dict(
        cwT=cwT,
        conv_b=np.asarray(conv_b, np.float32).reshape(256, 1),
        pcwT=pcwT,
        pc_b=np.asarray(pc_b, np.float32).reshape(256, 1),
        Wmm=Wmm,
        S2=S2,
        ident=np.eye(128, dtype=np.float32),
        w1T=np.ascontiguousarray(np.asarray(enc_w1, np.float32).T),   # [160, 512]
        b1=np.asarray(enc_b1, np.float32).reshape(512, 1),
        w2T=np.ascontiguousarray(np.pad(np.asarray(enc_w2, np.float32).T,
                                        ((0, 0), (0, 1)))),          # [512, 10]
        b2rep=np.tile(np.asarray(enc_b2, np.float32).reshape(1, 9), (B, 1)),
    )
    return shared, per_core


def kernel(**inputs):
    global _CACHED_NC
    if _CACHED_NC is None:
        _CACHED_NC = build()
    nc = _CACHED_NC
    inputs = {k: np.asarray(v, dtype=np.float32) for k, v in inputs.items()}
    shared, per_core = _host_prep(**inputs)
    in_maps = [dict(shared, im2col=per_core[c]) for c in range(N_CORES)]
    res = bass_utils.run_bass_kernel_spmd(nc, in_maps, core_ids=list(range(N_CORES)))
    out = np.concatenate([res.results[c]["out"] for c in range(N_CORES)], axis=0)
    return out


if __name__ == "__main__":
    import reference
    inputs = {k: np.asarray(v) for k, v in reference.setup_inputs().items()}
    got = kernel(**inputs)
    exp = np.asarray(reference.reference(**inputs))
    rel = np.abs(got - exp).max() / np.abs(exp).max()
    print("Relative error:", rel)


# revision 13
# speedup vs baseline: 1.0306x; 1.0259x over previous
"""CapsNet forward pass on 8 Trainium2 NeuronCores (Bass/Tile).

Data-parallel: batch 512 sharded 64/core; parameters replicated; the routing
b_ij batch-mean update is an AllReduce. Host-side prep is layout-only
(im2col of the input, weight transposes) - all FLOPs run on device.

Self-contained: hardcodes all shapes from the problem spec.
"""
import os
import numpy as np

import concourse.bacc as bacc
import concourse.tile as tile
from concourse import bass_utils, mybir

F32 = mybir.dt.float32
F32R = mybir.dt.float32r
AF = mybir.ActivationFunctionType
ALU = mybir.AluOpType
AXL = mybir.AxisListType

N_CORES = 8
B = 64            # batch per core
BH = 32           # batch per conv sub-pass
R = 1152          # num routes
C = 10            # num capsules
O = 16            # out dim
RI = R * 8        # 9216
CO = C * O        # 160
KT = RI // 128    # 72 k-tiles over (r,i)
NUM_ITERS = 3

DEBUG = bool(int(os.environ.get("BASS_CAPS_DEBUG", "0")))

_CACHED_NC = None


def build():
    nc = bacc.Bacc("TRN2", target_bir_lowering=False, debug=False,
                   num_devices=N_CORES)

    # ---------------- DRAM I/O ----------------
    d_im2col = nc.dram_tensor("im2col", [81, 20, B, 20], F32, kind="ExternalInput")
    d_cwT = nc.dram_tensor("cwT", [81, 256], F32, kind="ExternalInput")
    d_conv_b = nc.dram_tensor("conv_b", [256, 1], F32, kind="ExternalInput")
    d_pcwT = nc.dram_tensor("pcwT", [2, 81, 128, 2, 128], F32, kind="ExternalInput")
    d_pc_b = nc.dram_tensor("pc_b", [256, 1], F32, kind="ExternalInput")
    d_Wmm = nc.dram_tensor("Wmm", [RI, CO], F32, kind="ExternalInput")
    d_S2 = nc.dram_tensor("S2", [128, 16], F32, kind="ExternalInput")
    d_ident = nc.dram_tensor("ident", [128, 128], F32, kind="ExternalInput")
    d_w1T = nc.dram_tensor("w1T", [CO, 512], F32, kind="ExternalInput")
    d_b1 = nc.dram_tensor("b1", [512, 1], F32, kind="ExternalInput")
    d_w2T = nc.dram_tensor("w2T", [512, 10], F32, kind="ExternalInput")
    d_b2rep = nc.dram_tensor("b2rep", [B, 9], F32, kind="ExternalInput")
    d_out = nc.dram_tensor("out", [B, 9], F32, kind="ExternalOutput")

    dbg = {}
    if DEBUG:
        dbg["xact"] = nc.dram_tensor("dbg_xact", [2, 128, 20, BH, 20], F32, kind="ExternalOutput")
        dbg["x2s"] = nc.dram_tensor("dbg_x2s", [2, 128, 6, B, 6], F32, kind="ExternalOutput")
        dbg["u"] = nc.dram_tensor("dbg_u", [B, RI], F32, kind="ExternalOutput")
        dbg["s0T"] = nc.dram_tensor("dbg_s0T", [128, B], F32, kind="ExternalOutput")
        dbg["db0"] = nc.dram_tensor("dbg_db0", [C, R], F32, kind="ExternalOutput")
        dbg["c1T"] = nc.dram_tensor("dbg_c1T", [C, R], F32, kind="ExternalOutput")
        dbg["vT2"] = nc.dram_tensor("dbg_vT2", [CO, B], F32, kind="ExternalOutput")
        dbg["hT"] = nc.dram_tensor("dbg_hT", [128, 4, B], F32, kind="ExternalOutput")

    with (
        tile.TileContext(nc) as tc,
        tc.tile_pool(name="persist", bufs=1) as P,
        tc.tile_pool(name="dram", bufs=1, space="DRAM") as DR,
    ):
        # ====== constants / small weights ======
        cw = P.tile([81, 256], F32R, tag="cw")
        nc.gpsimd.dma_start(cw[:], d_cwT[:])
        cb0 = P.tile([128, 2], F32, tag="cb0")
        nc.sync.dma_start(cb0[:, 0:1], d_conv_b[0:128, :])
        nc.sync.dma_start(cb0[:, 1:2], d_conv_b[128:256, :])
        pb0 = P.tile([128, 2], F32, tag="pb0")
        nc.sync.dma_start(pb0[:, 0:1], d_pc_b[0:128, :])
        nc.sync.dma_start(pb0[:, 1:2], d_pc_b[128:256, :])
        ident = P.tile([128, 128], F32, tag="ident")
        nc.sync.dma_start(ident[:], d_ident[:])
        S2 = P.tile([128, 16], F32R, tag="S2")
        nc.gpsimd.dma_start(S2[:], d_S2[:])

        x2s = [P.tile([128, 6, B, 6], F32, tag=f"x2s{mt}", name=f"x2s{mt}") for mt in range(2)]

        # ================= conv phase (per batch half) =================
        with (
            tc.tile_pool(name="convsb", bufs=1) as CB,
            tc.tile_pool(name="imcpool", bufs=3) as IMC,
            tc.tile_pool(name="wstream", bufs=4) as WS,
            tc.tile_pool(name="ps_c1", bufs=2, space="PSUM") as PC1,
            tc.tile_pool(name="ps_c2", bufs=1, space="PSUM") as PC2,
        ):
            xact = [CB.tile([128, 20, BH, 20], F32R, tag=f"xact{mt}", name=f"xact{mt}")
                    for mt in range(2)]
            for bh in range(2):
                bsl = slice(bh * BH, (bh + 1) * BH)
                scope_c1 = nc.named_scope(f"conv1_bh{bh}")
                scope_c1.__enter__()
                # ---- conv1 ----
                for y in range(20):
                    imc = IMC.tile([81, BH, 20], F32R, tag="imc")
                    nc.gpsimd.dma_start(imc[:], d_im2col[:, y, bsl, :])
                    for mt in range(2):
                        ps = PC1.tile([128, BH * 20], F32, tag="c1ps")
                        for n0, n1 in ((0, 512), (512, 640)):
                            nc.tensor.matmul(
                                ps[:, n0:n1],
                                cw[:, mt * 128:(mt + 1) * 128],
                                imc[:].rearrange("p a b -> p (a b)")[:, n0:n1],
                                start=True, stop=True,
                            )
                        dst = xact[mt][:, y, :, :].rearrange("p a b -> p (a b)")
                        if mt == 0:
                            nc.vector.tensor_scalar(
                                dst, ps[:], cb0[:, 0:1], 0.0, ALU.add, ALU.max)
                        else:
                            nc.scalar.activation(
                                dst, ps[:], AF.Relu, bias=cb0[:, 1:2], scale=1.0)
                scope_c1.__exit__(None, None, None)
                if DEBUG and bh == 1:
                    for mt in range(2):
                        nc.sync.dma_start(dbg["xact"][mt], xact[mt][:].bitcast(F32))

                # ---- conv2 ----
                scope_c2 = nc.named_scope(f"conv2_bh{bh}")
                scope_c2.__enter__()
                for mt in range(2):
                    ps2 = [PC2.tile([128, 2, BH, 6], F32, tag=f"c2ps{j}", name=f"c2ps{j}")
                           for j in range(3)]
                    for dydx in range(81):
                        dy, dx = divmod(dydx, 9)
                        wt = WS.tile([128, 2, 128], F32R, tag="wt")
                        nc.gpsimd.dma_start(wt[:], d_pcwT[mt, dydx])
                        for kh in range(2):
                            for oyp in range(3):
                                # rows 4*oyp + 2j + dy, j in {0,1}
                                rhs = xact[kh][:, 4 * oyp + dy:4 * oyp + dy + 3:2,
                                               :, dx:dx + 11:2]
                                nc.tensor.matmul(
                                    ps2[oyp][:], wt[:, kh, :], rhs,
                                    start=(dydx == 0 and kh == 0),
                                    stop=(dydx == 80 and kh == 1),
                                )
                    for oyp in range(3):
                        nc.vector.tensor_scalar(
                            x2s[mt][:, 2 * oyp:2 * oyp + 2, bsl, :],
                            ps2[oyp][:], pb0[:, mt:mt + 1], None, ALU.add,
                        )
                scope_c2.__exit__(None, None, None)
        if DEBUG:
            for mt in range(2):
                nc.sync.dma_start(dbg["x2s"][mt], x2s[mt][:])

        # ================= u phase =================
        scope_u = nc.named_scope("u_phase")
        scope_u.__enter__()
        with tc.tile_pool(name="upool", bufs=1) as UP:
            u_byB = UP.tile([B, RI], F32R, tag="u_byB")
            uT = UP.tile([128, KT, B], F32R, tag="uT")

            with (
                tc.tile_pool(name="usc", bufs=1) as USC,
                tc.tile_pool(name="usc2", bufs=2) as USC2,
                tc.tile_pool(name="ps_tr", bufs=4, space="PSUM") as PTR,
            ):
                u_pre = USC.tile([B, RI], F32, tag="u_pre")
                for mt in range(2):
                    for oy in range(6):
                        for ox in range(6):
                            tp = PTR.tile([128, 128], F32, tag="tr")
                            nc.tensor.transpose(
                                tp[0:B, :], x2s[mt][:, oy, :, ox], ident[:])
                            dst = u_pre[:, mt * 4608 + oy * 6 + ox::36][:, 0:128]
                            nc.vector.tensor_copy(dst, tp[0:B, :])

                sn = USC.tile([B, R], F32, tag="sn")
                for ch in range(9):
                    sl = slice(ch * 1024, (ch + 1) * 1024)
                    sq = USC2.tile([B, 1024], F32, tag="sq")
                    nc.scalar.activation(sq[:], u_pre[:, sl], AF.Square)
                    nc.vector.tensor_reduce(
                        sn[:, ch * 128:(ch + 1) * 128],
                        sq[:].rearrange("p (g i) -> p g i", i=8),
                        AXL.X, ALU.add,
                    )
                fac = USC.tile([B, R], F32, tag="fac")
                rt = USC.tile([B, R], F32, tag="rt")
                nc.scalar.activation(rt[:], sn[:], AF.Sqrt)
                snp = USC.tile([B, R], F32, tag="snp")
                nc.vector.tensor_scalar(snp[:], sn[:], 1.0, None, ALU.add)
                rsnp = USC.tile([B, R], F32, tag="rsnp")
                nc.vector.reciprocal(rsnp[:], snp[:])
                nc.vector.tensor_tensor(fac[:], rt[:], rsnp[:], ALU.mult)
                nc.vector.tensor_tensor(
                    u_byB[:].rearrange("p (r i) -> p r i", i=8),
                    u_pre[:].rearrange("p (r i) -> p r i", i=8),
                    fac[:].unsqueeze(2).to_broadcast([B, R, 8]),
                    ALU.mult,
                )
                if DEBUG:
                    nc.sync.dma_start(dbg["u"][:], u_byB[:].bitcast(F32))
                for k in range(KT):
                    tp = PTR.tile([128, 128], F32, tag="tr")
                    nc.tensor.transpose(
                        tp[:, 0:B], u_byB[:, k * 128:(k + 1) * 128].bitcast(F32),
                        ident[0:B, 0:B])
                    nc.vector.tensor_copy(uT[:, k, :], tp[:, 0:B])

            # ================= routing phase =================
            vT0 = P.tile([128, B], F32R, tag="vT0")
            vT1 = P.tile([32, B], F32R, tag="vT1")
            with (
                tc.tile_pool(name="rt_big", bufs=1) as RB,
                tc.tile_pool(name="rt_sc", bufs=1) as RS,
                tc.tile_pool(name="rt_sc2", bufs=2) as RS2,
                tc.tile_pool(name="ps_st", bufs=1, space="PSUM") as PST,
                tc.tile_pool(name="ps_m", bufs=2, space="PSUM") as PSM,
                tc.tile_pool(name="ps_db", bufs=1, space="PSUM") as PDB,
                tc.tile_pool(name="ps_tp", bufs=1, space="PSUM") as PTP,
            ):
                Wmm = RB.tile([128, KT, CO], F32R, tag="Wmm")
                nc.gpsimd.dma_start(
                    Wmm[:], d_Wmm[:].rearrange("(k p) c -> p k c", p=128))
                W1mm = RB.tile([128, KT, CO], F32R, tag="W1mm")
                c_rep = RB.tile([128, KT, C], F32, tag="c_rep")
                bacc_t = RB.tile([C, R], F32, tag="bacc")
                c_T = RB.tile([C, R], F32, tag="c_T")
                c_byR = RB.tile([128, 9, C], F32, tag="c_byR")

                d_crdram = DR.tile([R, C], F32)
                d_red_in = DR.tile([C, R], F32)
                d_red_out = DR.tile([C, R], F32)

                scope_u.__exit__(None, None, None)
                for it in range(NUM_ITERS):
                    scope_it = nc.named_scope(f"route{it}")
                    scope_it.__enter__()
                    # ---- s_t = W'^T @ u ----
                    st0 = PST.tile([128, B], F32, tag="st0")
                    st1 = PST.tile([32, B], F32, tag="st1")
                    lhs = Wmm if it == 0 else W1mm
                    for k in range(KT):
                        nc.tensor.matmul(st0[:], lhs[:, k, 0:128], uT[:, k, :],
                                         start=(k == 0), stop=(k == KT - 1))
                    for k in range(KT):
                        nc.tensor.matmul(st1[:], lhs[:, k, 128:160], uT[:, k, :],
                                         start=(k == 0), stop=(k == KT - 1))
                    # ---- v = squash(s) elementwise ----
                    for half, (st, vt, np_) in enumerate(
                            ((st0, vT0, 128), (st1, vT1, 32))):
                        s_sb = RS2.tile([np_, B], F32, tag=f"s_sb{half}")
                        if it == 0:
                            nc.vector.tensor_scalar(
                                s_sb[:], st[:], 1.0 / R, None, ALU.mult)
                        else:
                            nc.vector.tensor_copy(s_sb[:], st[:])
                        if DEBUG and it == 0 and half == 0:
                            nc.sync.dma_start(dbg["s0T"][:], s_sb[:])
                        t2 = RS2.tile([np_, B], F32, tag=f"t2_{half}")
                        nc.vector.tensor_tensor(t2[:], s_sb[:], s_sb[:], ALU.mult)
                        num = RS2.tile([np_, B], F32, tag=f"num{half}")
                        nc.vector.tensor_tensor(num[:], t2[:], s_sb[:], ALU.mult)
                        rte = RS2.tile([np_, B], F32, tag=f"rte{half}")
                        nc.scalar.activation(rte[:], t2[:], AF.Sqrt)
                        den = RS2.tile([np_, B], F32, tag=f"den{half}")
                        nc.vector.tensor_scalar(den[:], t2[:], 1.0, None, ALU.add)
                        nc.vector.tensor_tensor(den[:], den[:], rte[:], ALU.mult)
                        rden = RS2.tile([np_, B], F32, tag=f"rden{half}")
                        nc.vector.reciprocal(rden[:], den[:])
                        nc.vector.tensor_tensor(vt[:], num[:], rden[:], ALU.mult)

                    if it == NUM_ITERS - 1:
                        scope_it.__exit__(None, None, None)
                        break

                    # ---- v -> [B, CO] ----
                    v_b = RS.tile([B, CO], F32R, tag="v_b")
                    tpv0 = PTP.tile([128, 128], F32, tag="tp")
                    nc.tensor.transpose(tpv0[0:B, :], vT0[:].bitcast(F32), ident[:])
                    nc.vector.tensor_copy(v_b[:, 0:128], tpv0[0:B, :])
                    tpv1 = PTP.tile([128, 128], F32, tag="tp")
                    nc.tensor.transpose(tpv1[0:B, 0:32], vT1[:].bitcast(F32), ident[0:32, 0:32])
                    nc.vector.tensor_copy(v_b[:, 128:160], tpv1[0:B, 0:32])

                    # ---- db^T[c, r] = sum_oi W*(u^T v)/Btot via per-ktile chain ----
                    db_ps = PDB.tile([C, R], F32, tag="db_ps")
                    for k in range(KT):
                        Mps = PSM.tile([128, CO], F32, tag="Mps")
                        nc.tensor.matmul(
                            Mps[:], u_byB[:, k * 128:(k + 1) * 128], v_b[:],
                            start=True, stop=True)
                        prod = RS2.tile([128, CO], F32, tag="prod")
                        nc.vector.tensor_tensor(
                            prod[:], Mps[:], Wmm[:, k, :], ALU.mult)
                        Tk = RS2.tile([128, C], F32R, tag="Tk")
                        with nc.allow_low_precision(reason="f32r rounding for PE"):
                            nc.vector.tensor_reduce(
                                Tk[:], prod[:].rearrange("p (c o) -> p c o", o=O),
                                AXL.X, ALU.add)
                        nc.tensor.matmul(
                            db_ps[:, k * 16:(k + 1) * 16], Tk[:], S2[:],
                            start=True, stop=True)
                    db_sb = RS.tile([C, R], F32, tag="db_sb")
                    nc.vector.tensor_copy(db_sb[:], db_ps[:])

                    # ---- AllReduce batch-mean across cores ----
                    nc.sync.dma_start(d_red_in[:], db_sb[:])
                    nc.gpsimd.collective_compute(
                        "AllReduce", ALU.add,
                        replica_groups=[list(range(N_CORES))],
                        ins=[d_red_in.opt()], outs=[d_red_out.opt()],
                    )
                    db_red = RS.tile([C, R], F32, tag="db_red")
                    nc.sync.dma_start(db_red[:], d_red_out[:])
                    if it == 0:
                        nc.vector.tensor_copy(bacc_t[:], db_red[:])
                        if DEBUG:
                            nc.sync.dma_start(dbg["db0"][:], bacc_t[:])
                    else:
                        nc.vector.tensor_tensor(
                            bacc_t[:], bacc_t[:], db_red[:], ALU.add)

                    # ---- c = softmax_r(b) on [C, R] ----
                    mx = RS.tile([C, 1], F32, tag="mx")
                    nc.vector.tensor_reduce(mx[:], bacc_t[:], AXL.X, ALU.max)
                    nmx = RS.tile([C, 1], F32, tag="nmx")
                    nc.vector.tensor_scalar(nmx[:], mx[:], -1.0, None, ALU.mult)
                    ex = RS.tile([C, R], F32, tag="ex")
                    nc.scalar.activation(ex[:], bacc_t[:], AF.Exp,
                                         bias=nmx[:], scale=1.0)
                    sm = RS.tile([C, 1], F32, tag="sm")
                    nc.vector.tensor_reduce(sm[:], ex[:], AXL.X, ALU.add)
                    rcp = RS.tile([C, 1], F32, tag="rcp")
                    nc.vector.reciprocal(rcp[:], sm[:])
                    nc.vector.tensor_scalar(c_T[:], ex[:], rcp[:], None, ALU.mult)
                    if DEBUG and it == 0:
                        nc.sync.dma_start(dbg["c1T"][:], c_T[:])

                    # ---- c_rep[(r16,i),(k,c)] via DRAM round trip ----
                    for rb in range(9):
                        tpc = PTP.tile([128, 128], F32, tag="tp")
                        nc.tensor.transpose(
                            tpc[:, 0:C], c_T[:, rb * 128:(rb + 1) * 128],
                            ident[0:C, 0:C])
                        nc.vector.tensor_copy(c_byR[:, rb, :], tpc[:, 0:C])
                    nc.sync.dma_start(
                        d_crdram[:].rearrange("(a p) c -> p a c", p=128),
                        c_byR[:])
                    for rlo in range(16):
                        nc.sync.dma_start(
                            c_rep[rlo * 8:(rlo + 1) * 8, :, :],
                            d_crdram[:].rearrange("(k rl) c -> rl k c", rl=16)[rlo]
                            .unsqueeze(0).to_broadcast([8, KT, C]),
                        )
                    # ---- W' = W * c (broadcast over o) ----
                    nc.vector.tensor_tensor(
                        W1mm[:].rearrange("p k (c o) -> p k c o", o=O),
                        Wmm[:].rearrange("p k (c o) -> p k c o", o=O),
                        c_rep[:].unsqueeze(3).to_broadcast([128, KT, C, O]),
                        ALU.mult,
                    )
                    scope_it.__exit__(None, None, None)

        # ================= encoder =================
        if DEBUG:
            nc.sync.dma_start(dbg["vT2"][0:128, :], vT0[:].bitcast(F32))
            nc.sync.dma_start(dbg["vT2"][128:160, :], vT1[:].bitcast(F32))
        scope_enc = nc.named_scope("encoder")
        scope_enc.__enter__()
        with (
            tc.tile_pool(name="enc", bufs=1) as EN,
            tc.tile_pool(name="enc2", bufs=2) as EN2,
            tc.tile_pool(name="ps_h", bufs=2, space="PSUM") as PSH,
            tc.tile_pool(name="ps_o", bufs=1, space="PSUM") as PSO,
        ):
            w1T0 = EN.tile([128, 512], F32R, tag="w1T0")
            nc.gpsimd.dma_start(w1T0[:], d_w1T[0:128, :])
            w1T1 = EN.tile([32, 512], F32R, tag="w1T1")
            nc.gpsimd.dma_start(w1T1[:], d_w1T[128:160, :])
            b1sb = EN.tile([128, 4], F32, tag="b1sb")
            for jt in range(4):
                nc.sync.dma_start(b1sb[:, jt:jt + 1],
                                  d_b1[jt * 128:(jt + 1) * 128, :])
            w2T = EN.tile([128, 4, 10], F32R, tag="w2T")
            nc.gpsimd.dma_start(
                w2T[:], d_w2T[:].rearrange("(j p) c -> p j c", p=128))
            b2sb = EN.tile([B, 9], F32, tag="b2sb")
            nc.sync.dma_start(b2sb[:], d_b2rep[:])

            hT = EN.tile([128, 4, B], F32R, tag="hT")
            for jt in range(4):
                hp = PSH.tile([128, B], F32, tag="hp")
                nc.tensor.matmul(hp[:], w1T0[:, jt * 128:(jt + 1) * 128], vT0[:],
                                 start=True, stop=False)
                nc.tensor.matmul(hp[:], w1T1[:, jt * 128:(jt + 1) * 128], vT1[:],
                                 start=False, stop=True)
                nc.scalar.activation(hT[:, jt, :], hp[:], AF.Relu,
                                     bias=b1sb[:, jt:jt + 1], scale=1.0)
            if DEBUG:
                nc.sync.dma_start(dbg["hT"][:], hT[:].bitcast(F32))

            op = PSO.tile([B, 10], F32, tag="op")
            for jt in range(4):
                nc.tensor.matmul(op[:], hT[:, jt, :], w2T[:, jt, :],
                                 start=(jt == 0), stop=(jt == 3))
            logit = EN2.tile([B, 9], F32, tag="logit")
            nc.vector.tensor_tensor(logit[:], op[:, 0:9], b2sb[:], ALU.add)
            out_sb = EN2.tile([B, 9], F32, tag="out_sb")
            nc.scalar.activation(out_sb[:], logit[:], AF.Sigmoid)
            nc.sync.dma_start(d_out[:], out_sb[:])
        scope_enc.__exit__(None, None, None)

    nc.compile()
    return nc


def _host_prep(data, conv_w, conv_b, pc_w, pc_b, W, enc_w1, enc_b1, enc_w2, enc_b2):
    """Layout-only host prep. Returns (shared_inputs, per_core_im2col)."""
    Bfull = data.shape[0]
    assert Bfull == N_CORES * B
    d = np.ascontiguousarray(data[:, 0])  # [512, 28, 28]
    sw = np.lib.stride_tricks.sliding_window_view(d, (9, 9), axis=(1, 2))
    # sw: [Bfull, 20, 20, 9, 9] -> (dy,dx,y,b,x)
    im2col_all = np.ascontiguousarray(sw.transpose(3, 4, 1, 0, 2)).reshape(
        81, 20, Bfull, 20)
    per_core = [np.ascontiguousarray(im2col_all[:, :, c * B:(c + 1) * B, :])
                for c in range(N_CORES)]

    cwT = np.ascontiguousarray(conv_w.reshape(256, 81).T)          # [81, 256]
    pcwT = np.ascontiguousarray(
        pc_w.reshape(2, 128, 2, 128, 81).transpose(0, 4, 3, 2, 1))  # [2,81,128,2,128]
    Wmm = np.ascontiguousarray(W.transpose(0, 3, 1, 2).reshape(RI, CO))
    S2 = np.zeros((128, 16), dtype=np.float32)
    for rlo in range(16):
        for i in range(8):
            S2[rlo * 8 + i, rlo] = 1.0 / (N_CORES * B)
    shared = dict(
        cwT=cwT,
        conv_b=np.asarray(conv_b, np.float32).reshape(256, 1),
        pcwT=pcwT,
        pc_b=np.asarray(pc_b, np.float32).reshape(256, 1),
        Wmm=Wmm,
        S2=S2,
        ident=np.eye(128, dtype=np.float32),
        w1T=np.ascontiguousarray(np.asarray(enc_w1, np.float32).T),   # [160, 512]
        b1=np.asarray(enc_b1, np.float32).reshape(512, 1),
        w2T=np.ascontiguousarray(np.pad(np.asarray(enc_w2, np.float32).T,
                                        ((0, 0), (0, 1)))),          # [512, 10]
        b2rep=np.tile(np.asarray(enc_b2, np.float32).reshape(1, 9), (B, 1)),
    )
    return shared, per_core


def kernel(**inputs):
    global _CACHED_NC
    if _CACHED_NC is None:
        _CACHED_NC = build()
    nc = _CACHED_NC
    inputs = {k: np.asarray(v, dtype=np.float32) for k, v in inputs.items()}
    shared, per_core = _host_prep(**inputs)
    in_maps = [dict(shared, im2col=per_core[c]) for c in range(N_CORES)]
    res = bass_utils.run_bass_kernel_spmd(nc, in_maps, core_ids=list(range(N_CORES)))
    out = np.concatenate([res.results[c]["out"] for c in range(N_CORES)], axis=0)
    return out


if __name__ == "__main__":
    import reference
    inputs = {k: np.asarray(v) for k, v in reference.setup_inputs().items()}
    got = kernel(**inputs)
    exp = np.asarray(reference.reference(**inputs))
    rel = np.abs(got - exp).max() / np.abs(exp).max()
    print("Relative error:", rel)


# revision 14
# speedup vs baseline: 1.1947x; 1.1592x over previous
"""CapsNet forward pass on 8 Trainium2 NeuronCores (Bass/Tile).

Data-parallel: batch 512 sharded 64/core; parameters replicated; the routing
b_ij batch-mean update is an AllReduce. Host-side prep is layout-only
(im2col of the input, weight transposes) - all FLOPs run on device.

Self-contained: hardcodes all shapes from the problem spec.
"""
import os
import numpy as np

import concourse.bacc as bacc
import concourse.tile as tile
from concourse import bass_utils, mybir

F32 = mybir.dt.float32
F32R = mybir.dt.float32r
BF16 = mybir.dt.bfloat16
AF = mybir.ActivationFunctionType
ALU = mybir.AluOpType
AXL = mybir.AxisListType

N_CORES = 8
B = 64            # batch per core
BH = 32           # batch per conv sub-pass
R = 1152          # num routes
C = 10            # num capsules
O = 16            # out dim
RI = R * 8        # 9216
CO = C * O        # 160
KT = RI // 128    # 72 k-tiles over (r,i)
NUM_ITERS = 3

DEBUG = bool(int(os.environ.get("BASS_CAPS_DEBUG", "0")))

_CACHED_NC = None


def build():
    nc = bacc.Bacc("TRN2", target_bir_lowering=False, debug=False,
                   num_devices=N_CORES)

    # ---------------- DRAM I/O ----------------
    d_im2col = nc.dram_tensor("im2col", [81, 20, B, 20], F32, kind="ExternalInput")
    d_cwT = nc.dram_tensor("cwT", [81, 256], F32, kind="ExternalInput")
    d_conv_b = nc.dram_tensor("conv_b", [256, 1], F32, kind="ExternalInput")
    d_pcwT = nc.dram_tensor("pcwT", [2, 81, 128, 2, 128], F32, kind="ExternalInput")
    d_pc_b = nc.dram_tensor("pc_b", [256, 1], F32, kind="ExternalInput")
    d_Wmm = nc.dram_tensor("Wmm", [RI, CO], F32, kind="ExternalInput")
    d_S2 = nc.dram_tensor("S2", [128, 16], F32, kind="ExternalInput")
    d_ident = nc.dram_tensor("ident", [128, 128], F32, kind="ExternalInput")
    d_w1T = nc.dram_tensor("w1T", [CO, 512], F32, kind="ExternalInput")
    d_b1 = nc.dram_tensor("b1", [512, 1], F32, kind="ExternalInput")
    d_w2T = nc.dram_tensor("w2T", [512, 10], F32, kind="ExternalInput")
    d_b2rep = nc.dram_tensor("b2rep", [B, 9], F32, kind="ExternalInput")
    d_out = nc.dram_tensor("out", [B, 9], F32, kind="ExternalOutput")

    dbg = {}
    if DEBUG:
        dbg["xact"] = nc.dram_tensor("dbg_xact", [2, 128, 20, BH, 20], F32, kind="ExternalOutput")
        dbg["x2s"] = nc.dram_tensor("dbg_x2s", [2, 128, 6, B, 6], F32, kind="ExternalOutput")
        dbg["u"] = nc.dram_tensor("dbg_u", [B, RI], F32, kind="ExternalOutput")
        dbg["s0T"] = nc.dram_tensor("dbg_s0T", [128, B], F32, kind="ExternalOutput")
        dbg["db0"] = nc.dram_tensor("dbg_db0", [C, R], F32, kind="ExternalOutput")
        dbg["c1T"] = nc.dram_tensor("dbg_c1T", [C, R], F32, kind="ExternalOutput")
        dbg["vT2"] = nc.dram_tensor("dbg_vT2", [CO, B], F32, kind="ExternalOutput")
        dbg["hT"] = nc.dram_tensor("dbg_hT", [128, 4, B], F32, kind="ExternalOutput")

    with (
        tile.TileContext(nc) as tc,
        tc.tile_pool(name="persist", bufs=1) as P,
        tc.tile_pool(name="dram", bufs=1, space="DRAM") as DR,
    ):
        # ====== constants / small weights ======
        cw = P.tile([81, 256], BF16, tag="cw")
        nc.gpsimd.dma_start(cw[:], d_cwT[:])
        cb0 = P.tile([128, 2], F32, tag="cb0")
        nc.sync.dma_start(cb0[:, 0:1], d_conv_b[0:128, :])
        nc.sync.dma_start(cb0[:, 1:2], d_conv_b[128:256, :])
        pb0 = P.tile([128, 2], F32, tag="pb0")
        nc.sync.dma_start(pb0[:, 0:1], d_pc_b[0:128, :])
        nc.sync.dma_start(pb0[:, 1:2], d_pc_b[128:256, :])
        ident = P.tile([128, 128], F32, tag="ident")
        nc.sync.dma_start(ident[:], d_ident[:])
        S2 = P.tile([128, 16], F32R, tag="S2")
        nc.gpsimd.dma_start(S2[:], d_S2[:])

        x2s = [P.tile([128, 6, B, 6], F32, tag=f"x2s{mt}", name=f"x2s{mt}") for mt in range(2)]

        # ================= conv phase (per batch half) =================
        with (
            tc.tile_pool(name="convsb", bufs=1) as CB,
            tc.tile_pool(name="imcpool", bufs=3) as IMC,
            tc.tile_pool(name="wstream", bufs=6) as WS,
            tc.tile_pool(name="ps_c1", bufs=2, space="PSUM") as PC1,
            tc.tile_pool(name="ps_c2", bufs=1, space="PSUM") as PC2,
        ):
            xact = [CB.tile([128, 20, BH, 20], BF16, tag=f"xact{mt}", name=f"xact{mt}")
                    for mt in range(2)]
            for bh in range(2):
                bsl = slice(bh * BH, (bh + 1) * BH)
                scope_c1 = nc.named_scope(f"conv1_bh{bh}")
                scope_c1.__enter__()
                # ---- conv1 ----
                for y in range(20):
                    imc = IMC.tile([81, BH, 20], BF16, tag="imc")
                    nc.gpsimd.dma_start(imc[:], d_im2col[:, y, bsl, :])
                    for mt in range(2):
                        ps = PC1.tile([128, BH * 20], F32, tag="c1ps")
                        for n0, n1 in ((0, 512), (512, 640)):
                            nc.tensor.matmul(
                                ps[:, n0:n1],
                                cw[:, mt * 128:(mt + 1) * 128],
                                imc[:].rearrange("p a b -> p (a b)")[:, n0:n1],
                                start=True, stop=True,
                            )
                        dst = xact[mt][:, y, :, :].rearrange("p a b -> p (a b)")
                        if mt == 0:
                            nc.vector.tensor_scalar(
                                dst, ps[:], cb0[:, 0:1], 0.0, ALU.add, ALU.max)
                        else:
                            nc.scalar.activation(
                                dst, ps[:], AF.Relu, bias=cb0[:, 1:2], scale=1.0)
                scope_c1.__exit__(None, None, None)
                if DEBUG and bh == 1:
                    for mt in range(2):
                        nc.gpsimd.dma_start(dbg["xact"][mt], xact[mt][:])

                # ---- conv2 ----
                scope_c2 = nc.named_scope(f"conv2_bh{bh}")
                scope_c2.__enter__()
                for mt in range(2):
                    ps2 = [PC2.tile([128, 2, BH, 6], F32, tag=f"c2ps{j}", name=f"c2ps{j}")
                           for j in range(3)]
                    for dydx in range(81):
                        dy, dx = divmod(dydx, 9)
                        wt = WS.tile([128, 2, 128], BF16, tag="wt")
                        nc.gpsimd.dma_start(wt[:], d_pcwT[mt, dydx])
                        for kh in range(2):
                            for oyp in range(3):
                                # rows 4*oyp + 2j + dy, j in {0,1}
                                rhs = xact[kh][:, 4 * oyp + dy:4 * oyp + dy + 3:2,
                                               :, dx:dx + 11:2]
                                nc.tensor.matmul(
                                    ps2[oyp][:], wt[:, kh, :], rhs,
                                    start=(dydx == 0 and kh == 0),
                                    stop=(dydx == 80 and kh == 1),
                                )
                    for oyp in range(3):
                        nc.vector.tensor_scalar(
                            x2s[mt][:, 2 * oyp:2 * oyp + 2, bsl, :],
                            ps2[oyp][:], pb0[:, mt:mt + 1], None, ALU.add,
                        )
                scope_c2.__exit__(None, None, None)
        if DEBUG:
            for mt in range(2):
                nc.sync.dma_start(dbg["x2s"][mt], x2s[mt][:])

        # ================= u phase =================
        scope_u = nc.named_scope("u_phase")
        scope_u.__enter__()
        with tc.tile_pool(name="upool", bufs=1) as UP:
            u_byB = UP.tile([B, RI], F32R, tag="u_byB")
            uT = UP.tile([128, KT, B], F32R, tag="uT")

            with (
                tc.tile_pool(name="usc", bufs=1) as USC,
                tc.tile_pool(name="usc2", bufs=2) as USC2,
                tc.tile_pool(name="ps_tr", bufs=4, space="PSUM") as PTR,
            ):
                u_pre = USC.tile([B, RI], F32, tag="u_pre")
                for mt in range(2):
                    for oy in range(6):
                        for ox in range(6):
                            tp = PTR.tile([128, 128], F32, tag="tr")
                            nc.tensor.transpose(
                                tp[0:B, :], x2s[mt][:, oy, :, ox], ident[:])
                            dst = u_pre[:, mt * 4608 + oy * 6 + ox::36][:, 0:128]
                            nc.vector.tensor_copy(dst, tp[0:B, :])

                sn = USC.tile([B, R], F32, tag="sn")
                for ch in range(9):
                    sl = slice(ch * 1024, (ch + 1) * 1024)
                    sq = USC2.tile([B, 1024], F32, tag="sq")
                    nc.scalar.activation(sq[:], u_pre[:, sl], AF.Square)
                    nc.vector.tensor_reduce(
                        sn[:, ch * 128:(ch + 1) * 128],
                        sq[:].rearrange("p (g i) -> p g i", i=8),
                        AXL.X, ALU.add,
                    )
                fac = USC.tile([B, R], F32, tag="fac")
                rt = USC.tile([B, R], F32, tag="rt")
                nc.scalar.activation(rt[:], sn[:], AF.Sqrt)
                snp = USC.tile([B, R], F32, tag="snp")
                nc.vector.tensor_scalar(snp[:], sn[:], 1.0, None, ALU.add)
                rsnp = USC.tile([B, R], F32, tag="rsnp")
                nc.vector.reciprocal(rsnp[:], snp[:])
                nc.vector.tensor_tensor(fac[:], rt[:], rsnp[:], ALU.mult)
                nc.vector.tensor_tensor(
                    u_byB[:].rearrange("p (r i) -> p r i", i=8),
                    u_pre[:].rearrange("p (r i) -> p r i", i=8),
                    fac[:].unsqueeze(2).to_broadcast([B, R, 8]),
                    ALU.mult,
                )
                if DEBUG:
                    nc.sync.dma_start(dbg["u"][:], u_byB[:].bitcast(F32))
                for k in range(KT):
                    tp = PTR.tile([128, 128], F32, tag="tr")
                    nc.tensor.transpose(
                        tp[:, 0:B], u_byB[:, k * 128:(k + 1) * 128].bitcast(F32),
                        ident[0:B, 0:B])
                    nc.vector.tensor_copy(uT[:, k, :], tp[:, 0:B])

            # ================= routing phase =================
            vT0 = P.tile([128, B], F32R, tag="vT0")
            vT1 = P.tile([32, B], F32R, tag="vT1")
            with (
                tc.tile_pool(name="rt_big", bufs=1) as RB,
                tc.tile_pool(name="rt_sc", bufs=1) as RS,
                tc.tile_pool(name="rt_sc2", bufs=2) as RS2,
                tc.tile_pool(name="ps_st", bufs=1, space="PSUM") as PST,
                tc.tile_pool(name="ps_m", bufs=2, space="PSUM") as PSM,
                tc.tile_pool(name="ps_db", bufs=1, space="PSUM") as PDB,
                tc.tile_pool(name="ps_tp", bufs=1, space="PSUM") as PTP,
            ):
                Wmm = RB.tile([128, KT, CO], F32R, tag="Wmm")
                nc.gpsimd.dma_start(
                    Wmm[:], d_Wmm[:].rearrange("(k p) c -> p k c", p=128))
                W1mm = RB.tile([128, KT, CO], F32R, tag="W1mm")
                c_rep = RB.tile([128, KT, C], F32, tag="c_rep")
                bacc_t = RB.tile([C, R], F32, tag="bacc")
                c_T = RB.tile([C, R], F32, tag="c_T")
                c_byR = RB.tile([128, 9, C], F32, tag="c_byR")

                d_crdram = DR.tile([R, C], F32)
                d_red_in = DR.tile([C, R], F32)
                d_red_out = DR.tile([C, R], F32)

                scope_u.__exit__(None, None, None)
                for it in range(NUM_ITERS):
                    scope_it = nc.named_scope(f"route{it}")
                    scope_it.__enter__()
                    # ---- s_t = W'^T @ u ----
                    st0 = PST.tile([128, B], F32, tag="st0")
                    st1 = PST.tile([32, B], F32, tag="st1")
                    lhs = Wmm if it == 0 else W1mm
                    for k in range(KT):
                        nc.tensor.matmul(st0[:], lhs[:, k, 0:128], uT[:, k, :],
                                         start=(k == 0), stop=(k == KT - 1))
                    for k in range(KT):
                        nc.tensor.matmul(st1[:], lhs[:, k, 128:160], uT[:, k, :],
                                         start=(k == 0), stop=(k == KT - 1))
                    # ---- v = squash(s) elementwise ----
                    for half, (st, vt, np_) in enumerate(
                            ((st0, vT0, 128), (st1, vT1, 32))):
                        s_sb = RS2.tile([np_, B], F32, tag=f"s_sb{half}")
                        if it == 0:
                            nc.vector.tensor_scalar(
                                s_sb[:], st[:], 1.0 / R, None, ALU.mult)
                        else:
                            nc.vector.tensor_copy(s_sb[:], st[:])
                        if DEBUG and it == 0 and half == 0:
                            nc.sync.dma_start(dbg["s0T"][:], s_sb[:])
                        t2 = RS2.tile([np_, B], F32, tag=f"t2_{half}")
                        nc.vector.tensor_tensor(t2[:], s_sb[:], s_sb[:], ALU.mult)
                        num = RS2.tile([np_, B], F32, tag=f"num{half}")
                        nc.vector.tensor_tensor(num[:], t2[:], s_sb[:], ALU.mult)
                        rte = RS2.tile([np_, B], F32, tag=f"rte{half}")
                        nc.scalar.activation(rte[:], t2[:], AF.Sqrt)
                        den = RS2.tile([np_, B], F32, tag=f"den{half}")
                        nc.vector.tensor_scalar(den[:], t2[:], 1.0, None, ALU.add)
                        nc.vector.tensor_tensor(den[:], den[:], rte[:], ALU.mult)
                        rden = RS2.tile([np_, B], F32, tag=f"rden{half}")
                        nc.vector.reciprocal(rden[:], den[:])
                        nc.vector.tensor_tensor(vt[:], num[:], rden[:], ALU.mult)

                    if it == NUM_ITERS - 1:
                        scope_it.__exit__(None, None, None)
                        break

                    # ---- v -> [B, CO] ----
                    v_b = RS.tile([B, CO], F32R, tag="v_b")
                    tpv0 = PTP.tile([128, 128], F32, tag="tp")
                    nc.tensor.transpose(tpv0[0:B, :], vT0[:].bitcast(F32), ident[:])
                    nc.vector.tensor_copy(v_b[:, 0:128], tpv0[0:B, :])
                    tpv1 = PTP.tile([128, 128], F32, tag="tp")
                    nc.tensor.transpose(tpv1[0:B, 0:32], vT1[:].bitcast(F32), ident[0:32, 0:32])
                    nc.vector.tensor_copy(v_b[:, 128:160], tpv1[0:B, 0:32])

                    # ---- db^T[c, r] = sum_oi W*(u^T v)/Btot via per-ktile chain ----
                    db_ps = PDB.tile([C, R], F32, tag="db_ps")
                    for k in range(KT):
                        Mps = PSM.tile([128, CO], F32, tag="Mps")
                        nc.tensor.matmul(
                            Mps[:], u_byB[:, k * 128:(k + 1) * 128], v_b[:],
                            start=True, stop=True)
                        prod = RS2.tile([128, CO], F32, tag="prod")
                        nc.vector.tensor_tensor(
                            prod[:], Mps[:], Wmm[:, k, :], ALU.mult)
                        Tk = RS2.tile([128, C], F32R, tag="Tk")
                        with nc.allow_low_precision(reason="f32r rounding for PE"):
                            nc.vector.tensor_reduce(
                                Tk[:], prod[:].rearrange("p (c o) -> p c o", o=O),
                                AXL.X, ALU.add)
                        nc.tensor.matmul(
                            db_ps[:, k * 16:(k + 1) * 16], Tk[:], S2[:],
                            start=True, stop=True)
                    db_sb = RS.tile([C, R], F32, tag="db_sb")
                    nc.vector.tensor_copy(db_sb[:], db_ps[:])

                    # ---- AllReduce batch-mean across cores ----
                    nc.sync.dma_start(d_red_in[:], db_sb[:])
                    nc.gpsimd.collective_compute(
                        "AllReduce", ALU.add,
                        replica_groups=[list(range(N_CORES))],
                        ins=[d_red_in.opt()], outs=[d_red_out.opt()],
                    )
                    db_red = RS.tile([C, R], F32, tag="db_red")
                    nc.sync.dma_start(db_red[:], d_red_out[:])
                    if it == 0:
                        nc.vector.tensor_copy(bacc_t[:], db_red[:])
                        if DEBUG:
                            nc.sync.dma_start(dbg["db0"][:], bacc_t[:])
                    else:
                        nc.vector.tensor_tensor(
                            bacc_t[:], bacc_t[:], db_red[:], ALU.add)

                    # ---- c = softmax_r(b) on [C, R] ----
                    mx = RS.tile([C, 1], F32, tag="mx")
                    nc.vector.tensor_reduce(mx[:], bacc_t[:], AXL.X, ALU.max)
                    nmx = RS.tile([C, 1], F32, tag="nmx")
                    nc.vector.tensor_scalar(nmx[:], mx[:], -1.0, None, ALU.mult)
                    ex = RS.tile([C, R], F32, tag="ex")
                    nc.scalar.activation(ex[:], bacc_t[:], AF.Exp,
                                         bias=nmx[:], scale=1.0)
                    sm = RS.tile([C, 1], F32, tag="sm")
                    nc.vector.tensor_reduce(sm[:], ex[:], AXL.X, ALU.add)
                    rcp = RS.tile([C, 1], F32, tag="rcp")
                    nc.vector.reciprocal(rcp[:], sm[:])
                    nc.vector.tensor_scalar(c_T[:], ex[:], rcp[:], None, ALU.mult)
                    if DEBUG and it == 0:
                        nc.sync.dma_start(dbg["c1T"][:], c_T[:])

                    # ---- c_rep[(r16,i),(k,c)] via DRAM round trip ----
                    for rb in range(9):
                        tpc = PTP.tile([128, 128], F32, tag="tp")
                        nc.tensor.transpose(
                            tpc[:, 0:C], c_T[:, rb * 128:(rb + 1) * 128],
                            ident[0:C, 0:C])
                        nc.vector.tensor_copy(c_byR[:, rb, :], tpc[:, 0:C])
                    nc.sync.dma_start(
                        d_crdram[:].rearrange("(a p) c -> p a c", p=128),
                        c_byR[:])
                    for rlo in range(16):
                        nc.sync.dma_start(
                            c_rep[rlo * 8:(rlo + 1) * 8, :, :],
                            d_crdram[:].rearrange("(k rl) c -> rl k c", rl=16)[rlo]
                            .unsqueeze(0).to_broadcast([8, KT, C]),
                        )
                    # ---- W' = W * c (broadcast over o) ----
                    nc.vector.tensor_tensor(
                        W1mm[:].rearrange("p k (c o) -> p k c o", o=O),
                        Wmm[:].rearrange("p k (c o) -> p k c o", o=O),
                        c_rep[:].unsqueeze(3).to_broadcast([128, KT, C, O]),
                        ALU.mult,
                    )
                    scope_it.__exit__(None, None, None)

        # ================= encoder =================
        if DEBUG:
            nc.sync.dma_start(dbg["vT2"][0:128, :], vT0[:].bitcast(F32))
            nc.sync.dma_start(dbg["vT2"][128:160, :], vT1[:].bitcast(F32))
        scope_enc = nc.named_scope("encoder")
        scope_enc.__enter__()
        with (
            tc.tile_pool(name="enc", bufs=1) as EN,
            tc.tile_pool(name="enc2", bufs=2) as EN2,
            tc.tile_pool(name="ps_h", bufs=2, space="PSUM") as PSH,
            tc.tile_pool(name="ps_o", bufs=1, space="PSUM") as PSO,
        ):
            w1T0 = EN.tile([128, 512], F32R, tag="w1T0")
            nc.gpsimd.dma_start(w1T0[:], d_w1T[0:128, :])
            w1T1 = EN.tile([32, 512], F32R, tag="w1T1")
            nc.gpsimd.dma_start(w1T1[:], d_w1T[128:160, :])
            b1sb = EN.tile([128, 4], F32, tag="b1sb")
            for jt in range(4):
                nc.sync.dma_start(b1sb[:, jt:jt + 1],
                                  d_b1[jt * 128:(jt + 1) * 128, :])
            w2T = EN.tile([128, 4, 10], F32R, tag="w2T")
            nc.gpsimd.dma_start(
                w2T[:], d_w2T[:].rearrange("(j p) c -> p j c", p=128))
            b2sb = EN.tile([B, 9], F32, tag="b2sb")
            nc.sync.dma_start(b2sb[:], d_b2rep[:])

            hT = EN.tile([128, 4, B], F32R, tag="hT")
            for jt in range(4):
                hp = PSH.tile([128, B], F32, tag="hp")
                nc.tensor.matmul(hp[:], w1T0[:, jt * 128:(jt + 1) * 128], vT0[:],
                                 start=True, stop=False)
                nc.tensor.matmul(hp[:], w1T1[:, jt * 128:(jt + 1) * 128], vT1[:],
                                 start=False, stop=True)
                nc.scalar.activation(hT[:, jt, :], hp[:], AF.Relu,
                                     bias=b1sb[:, jt:jt + 1], scale=1.0)
            if DEBUG:
                nc.sync.dma_start(dbg["hT"][:], hT[:].bitcast(F32))

            op = PSO.tile([B, 10], F32, tag="op")
            for jt in range(4):
                nc.tensor.matmul(op[:], hT[:, jt, :], w2T[:, jt, :],
                                 start=(jt == 0), stop=(jt == 3))
            logit = EN2.tile([B, 9], F32, tag="logit")
            nc.vector.tensor_tensor(logit[:], op[:, 0:9], b2sb[:], ALU.add)
            out_sb = EN2.tile([B, 9], F32, tag="out_sb")
            nc.scalar.activation(out_sb[:], logit[:], AF.Sigmoid)
            nc.sync.dma_start(d_out[:], out_sb[:])
        scope_enc.__exit__(None, None, None)

    nc.compile()
    return nc


def _host_prep(data, conv_w, conv_b, pc_w, pc_b, W, enc_w1, enc_b1, enc_w2, enc_b2):
    """Layout-only host prep. Returns (shared_inputs, per_core_im2col)."""
    Bfull = data.shape[0]
    assert Bfull == N_CORES * B
    d = np.ascontiguousarray(data[:, 0])  # [512, 28, 28]
    sw = np.lib.stride_tricks.sliding_window_view(d, (9, 9), axis=(1, 2))
    # sw: [Bfull, 20, 20, 9, 9] -> (dy,dx,y,b,x)
    im2col_all = np.ascontiguousarray(sw.transpose(3, 4, 1, 0, 2)).reshape(
        81, 20, Bfull, 20)
    per_core = [np.ascontiguousarray(im2col_all[:, :, c * B:(c + 1) * B, :])
                for c in range(N_CORES)]

    cwT = np.ascontiguousarray(conv_w.reshape(256, 81).T)          # [81, 256]
    pcwT = np.ascontiguousarray(
        pc_w.reshape(2, 128, 2, 128, 81).transpose(0, 4, 3, 2, 1))  # [2,81,128,2,128]
    Wmm = np.ascontiguousarray(W.transpose(0, 3, 1, 2).reshape(RI, CO))
    S2 = np.zeros((128, 16), dtype=np.float32)
    for rlo in range(16):
        for i in range(8):
            S2[rlo * 8 + i, rlo] = 1.0 / (N_CORES * B)
    shared = dict(
        cwT=cwT,
        conv_b=np.asarray(conv_b, np.float32).reshape(256, 1),
        pcwT=pcwT,
        pc_b=np.asarray(pc_b, np.float32).reshape(256, 1),
        Wmm=Wmm,
        S2=S2,
        ident=np.eye(128, dtype=np.float32),
        w1T=np.ascontiguousarray(np.asarray(enc_w1, np.float32).T),   # [160, 512]
        b1=np.asarray(enc_b1, np.float32).reshape(512, 1),
        w2T=np.ascontiguousarray(np.pad(np.asarray(enc_w2, np.float32).T,
                                        ((0, 0), (0, 1)))),          # [512, 10]
        b2rep=np.tile(np.asarray(enc_b2, np.float32).reshape(1, 9), (B, 1)),
    )
    return shared, per_core


def kernel(**inputs):
    global _CACHED_NC
    if _CACHED_NC is None:
        _CACHED_NC = build()
    nc = _CACHED_NC
    inputs = {k: np.asarray(v, dtype=np.float32) for k, v in inputs.items()}
    shared, per_core = _host_prep(**inputs)
    in_maps = [dict(shared, im2col=per_core[c]) for c in range(N_CORES)]
    res = bass_utils.run_bass_kernel_spmd(nc, in_maps, core_ids=list(range(N_CORES)))
    out = np.concatenate([res.results[c]["out"] for c in range(N_CORES)], axis=0)
    return out


if __name__ == "__main__":
    import reference
    inputs = {k: np.asarray(v) for k, v in reference.setup_inputs().items()}
    got = kernel(**inputs)
    exp = np.asarray(reference.reference(**inputs))
    rel = np.abs(got - exp).max() / np.abs(exp).max()
    print("Relative error:", rel)
